# revision 10
# baseline (speedup 1.0000x reference)
"""ALIGNN edge-gated message passing on 8 Trainium2 NeuronCores.

Strategy: edges partitioned by dst-block across cores (no collectives).
Each core receives host-prepared, per-core data:
  - a compacted node table (only nodes referenced as src by its edges)
  - its own 98 node-blocks (128 nodes each) in load-balanced "slot" order
  - its edge slice in canonical (slot, window, tile) order
Device work per core:
  phase A : PE matmuls node_feats -> T1=[e_src+bias | Bh] (f32, compact rows)
            and T2x=[e_dst | x_lin] (bf16, own blocks)
  phase B : per 128-edge tile: dma_gather T1 rows by src (int16 windows),
            one-hot(dst_local) via is_equal, m = ef@W_eg + onehot.T@T2blk
            (+ gathered, DVE), sigma = sigmoid(m), segment-sum via one-hot
            matmul accumulation in PSUM, y = ef + silu(LN(m)) via PE
            transpose + DVE add
  phase D : per slot: h = ssh/(ss+1e-6); x = nf + silu(LN(x_lin + h))
"""
import sys

if '/opt/trn_rl_repo' not in sys.path:
    sys.path.insert(0, '/opt/trn_rl_repo')

import numpy as np

H = 96
LN_EPS = 1e-5
NCORES = 8
P = 128
WMAX = 32640          # gather window rows (<= int16 max, mult of 128)
PHASES = 'ABD'        # debug bisect: which phases to build
BLEVEL = 9            # debug bisect: phase-B sub-level
DO_GATHER = True
DO_LOADS = True


# ----------------------------------------------------------------------------
# host-side plan
# ----------------------------------------------------------------------------

def build_plan(src, dst, N):
    E = src.shape[0]
    n_blocks_real = (N + P - 1) // P
    n_blocks = ((n_blocks_real + NCORES - 1) // NCORES) * NCORES
    S = n_blocks // NCORES              # slots per core
    N_pad = n_blocks * P

    blk_of_edge = dst // P
    blk_counts = np.bincount(blk_of_edge, minlength=n_blocks)

    # balanced assignment: sort blocks by count desc, greedily fill cores
    order = np.argsort(-blk_counts, kind='stable')
    core_load = np.zeros(NCORES, dtype=np.int64)
    core_nblk = np.zeros(NCORES, dtype=np.int64)
    blk_core = np.zeros(n_blocks, dtype=np.int64)
    for b in order:
        cands = np.where(core_nblk < S)[0]
        c = cands[np.argmin(core_load[cands])]
        blk_core[b] = c
        core_load[c] += blk_counts[b]
        core_nblk[c] += 1

    # per-core slot order: blocks sorted by count desc
    slot_block = np.zeros((NCORES, S), dtype=np.int64)   # slot -> block id
    for c in range(NCORES):
        blks = np.where(blk_core == c)[0]
        blks = blks[np.argsort(-blk_counts[blks], kind='stable')]
        slot_block[c] = blks

    # per-core edges grouped by slot
    edge_core = blk_core[blk_of_edge]
    slot_of_block = np.zeros(n_blocks, dtype=np.int64)
    for c in range(NCORES):
        slot_of_block[slot_block[c]] = np.arange(S)
    edge_slot = slot_of_block[blk_of_edge]

    # compacted src table per core
    srclist = []                        # core -> sorted unique srcs
    for c in range(NCORES):
        u = np.unique(src[edge_core == c])
        srclist.append(u)
    E_TBL = ((max(len(u) for u in srclist) + P - 1) // P) * P
    n_win = max(1, (E_TBL + WMAX - 1) // WMAX)
    WSZ = ((E_TBL // n_win + P - 1) // P) * P
    assert WSZ <= 32767

    # src position within the compact table, per core
    src_pos = np.zeros((NCORES, E), dtype=np.int64)   # only valid on own rows
    for c in range(NCORES):
        m = edge_core == c
        src_pos[c, m] = np.searchsorted(srclist[c], src[m])

    # group = (slot, window); count per (core, slot, window)
    cnt = np.zeros((NCORES, S, n_win), dtype=np.int64)
    for c in range(NCORES):
        m = edge_core == c
        w = src_pos[c, m] // WSZ
        np.add.at(cnt[c], (edge_slot[m], w), 1)
    tiles_sw = np.maximum(np.ceil(cnt / P).astype(np.int64).max(axis=0), 0)
    tiles_sw[:, 0] = np.maximum(tiles_sw[:, 0], 1)   # every slot >=1 tile
    TT = int(tiles_sw.sum())
    E_pad = TT * P

    # schedule: list of (slot, window, ntiles, tile_offset)
    sched = []
    off = 0
    for s in range(S):
        for w in range(n_win):
            t = int(tiles_sw[s, w])
            if t:
                sched.append((s, w, t, off))
                off += t

    # canonical per-core edge placement
    # pos_in_stream for edge e on its core; dummies fill the rest
    canon_edge = np.full((NCORES, E_pad), -1, dtype=np.int64)
    for c in range(NCORES):
        m = np.where(edge_core == c)[0]
        w = src_pos[c, m] // WSZ
        key = edge_slot[m] * n_win + w
        ordr = np.argsort(key, kind='stable')
        me, ke = m[ordr], key[ordr]
        # group boundaries in sorted list
        group_off = {(s_, w_): o_ * P for (s_, w_, t_, o_) in sched}
        pos = np.zeros(len(me), dtype=np.int64)
        start = 0
        for k in np.unique(ke):
            cnt_k = int((ke == k).sum())
            s_, w_ = divmod(int(k), n_win)
            base = group_off[(s_, w_)]
            pos[start:start + cnt_k] = base + np.arange(cnt_k)
            start += cnt_k
        canon_edge[c, pos] = me
    return dict(
        N_pad=N_pad, n_blocks=n_blocks, S=S, E_TBL=E_TBL, n_win=n_win,
        WSZ=WSZ, TT=TT, E_pad=E_pad, sched=sched, slot_block=slot_block,
        srclist=srclist, src_pos=src_pos, canon_edge=canon_edge,
    )


def build_inputs(plan, inputs):
    node_feats = np.asarray(inputs['node_feats'], np.float32)
    edge_feats = np.asarray(inputs['edge_feats'], np.float32)
    src = np.asarray(inputs['src'])
    dst = np.asarray(inputs['dst'])
    N = node_feats.shape[0]

    tp = (np.asarray(inputs['time_feats'], np.float32) @
          np.asarray(inputs['W_tp'], np.float32) +
          np.asarray(inputs['b_tp'], np.float32))[0]
    bias_src = np.asarray(inputs['b_sg'], np.float32) + tp + \
        np.asarray(inputs['b_eg'], np.float32)

    W1b = np.concatenate([
        np.concatenate([inputs['W_sg'], inputs['W_du']], axis=1),
        np.concatenate([bias_src, inputs['b_du']])[None, :],
    ], axis=0).astype(np.float32)                      # [97, 192]
    W2b = np.concatenate([
        np.concatenate([inputs['W_dg'], inputs['W_su']], axis=1),
        np.concatenate([inputs['b_dg'], inputs['b_su']])[None, :],
    ], axis=0).astype(np.float32)                      # [97, 192]

    S, E_TBL, E_pad, TT = plan['S'], plan['E_TBL'], plan['E_pad'], plan['TT']
    nf_pad = np.zeros((plan['N_pad'], H), np.float32)
    nf_pad[:N] = node_feats

    iota = np.tile(np.arange(P, dtype=np.float32), (P, 1))
    ident = np.eye(P, dtype=np.float32)

    in_maps = []
    for c in range(NCORES):
        u = plan['srclist'][c]
        nftc = np.zeros((97, E_TBL), np.float32)
        nftc[:H, :len(u)] = node_feats[u].T
        nftc[96, :] = 1.0

        blocks = plan['slot_block'][c]
        own = nf_pad.reshape(-1, P, H)[blocks]          # [S, 128, 96]
        own_flat = own.reshape(S * P, H)
        nfbT = np.zeros((97, S * P), np.float32)
        nfbT[:H] = own_flat.T
        nfbT[96] = 1.0

        canon = plan['canon_edge'][c]
        real = canon >= 0
        efT = np.zeros((H, E_pad), np.float32)
        efT[:, real] = edge_feats[canon[real]].T

        dstloc = np.full(E_pad, -1.0, np.float32)
        dstloc[real] = (dst[canon[real]] % P).astype(np.float32)
        dstloc = dstloc.reshape(TT, P).T.copy()         # [128, TT]

        gpos = np.zeros(E_pad, np.int64)
        gpos[real] = plan['src_pos'][c, canon[real]] % plan['WSZ']
        gidx = np.zeros((16, E_pad // 16), np.int16)
        idx_lin = np.arange(E_pad)
        gidx[idx_lin % 16, idx_lin // 16] = gpos.astype(np.int16)
        gidx = np.tile(gidx, (8, 1))                    # [128, E_pad/16]

        in_maps.append({
            'nftc': nftc.astype(np.float32),
            'nfbT': nfbT.astype(np.float32),
            'w1b': W1b, 'w2b': W2b,
            'weg': np.asarray(inputs['W_eg'], np.float32),
            'efT': efT,
            'dstloc': dstloc,
            'gidx': gidx,
            'iota': iota,
            'ident': ident,
            'nfb': own_flat,
        })
    return in_maps


# ----------------------------------------------------------------------------
# device kernel
# ----------------------------------------------------------------------------

def build_kernel(plan):
    import concourse.bacc as bacc
    import concourse.bass as bass
    import concourse.mybir as mybir
    import concourse.tile as tile

    f32, bf16, i16 = mybir.dt.float32, mybir.dt.bfloat16, mybir.dt.int16
    AF = mybir.ActivationFunctionType
    ALU = mybir.AluOpType

    S, E_TBL, E_pad, TT = plan['S'], plan['E_TBL'], plan['E_pad'], plan['TT']
    n_win, WSZ = plan['n_win'], plan['WSZ']
    sched = plan['sched']
    NB = S * P                                       # own nodes per core

    nc = bacc.Bacc()
    dp = nc.declare_dram_parameter
    nftc = dp('nftc', [97, E_TBL], f32, isOutput=False)
    nfbT = dp('nfbT', [97, NB], f32, isOutput=False)
    w1b = dp('w1b', [97, 192], f32, isOutput=False)
    w2b = dp('w2b', [97, 192], f32, isOutput=False)
    weg = dp('weg', [H, H], f32, isOutput=False)
    efT = dp('efT', [H, E_pad], f32, isOutput=False)
    dstloc = dp('dstloc', [P, TT], f32, isOutput=False)
    gidx = dp('gidx', [P, E_pad // 16], i16, isOutput=False)
    iota = dp('iota', [P, P], f32, isOutput=False)
    ident = dp('ident', [P, P], f32, isOutput=False)
    nfb = dp('nfb', [NB, H], f32, isOutput=False)
    yT = dp('yT', [H, E_pad], f32, isOutput=True)
    xout = dp('xout', [NB, H], f32, isOutput=True)

    t1c = nc.dram_tensor('t1c', [E_TBL, 192], f32)
    t2x = nc.dram_tensor('t2x', [NB, 192], bf16)

    with tile.TileContext(nc) as tc:
        with (
            tc.tile_pool(name='const', bufs=1) as cpool,
            tc.tile_pool(name='wts', bufs=1) as wpool,
            tc.tile_pool(name='io', bufs=3) as iop,
            tc.tile_pool(name='work', bufs=3) as wk,
            tc.tile_pool(name='ps', bufs=2, space='PSUM') as pp,
            tc.tile_pool(name='ps_sum', bufs=2, space='PSUM') as pps,
        ):
            # ---- constants ----
            iota_sb = cpool.tile([P, P], f32, tag='iota')
            nc.sync.dma_start(out=iota_sb[:], in_=iota[:])
            id_bf = cpool.tile([P, P], bf16, tag='idb')
            nc.gpsimd.dma_start(out=id_bf[:], in_=ident[:])   # cast f32->bf16
            w1_sb = wpool.tile([97, 192], bf16, tag='w1')
            nc.gpsimd.dma_start(out=w1_sb[:], in_=w1b[:])
            w2_sb = wpool.tile([97, 192], bf16, tag='w2')
            nc.gpsimd.dma_start(out=w2_sb[:], in_=w2b[:])
            weg_sb = wpool.tile([H, H], bf16, tag='weg')
            nc.gpsimd.dma_start(out=weg_sb[:], in_=weg[:])
            idx_all = cpool.tile([P, E_pad // 16], i16, tag='gidx')
            nc.sync.dma_start(out=idx_all[:], in_=gidx[:])
            dl_all = cpool.tile([P, TT], f32, tag='dstloc')
            nc.sync.dma_start(out=dl_all[:], in_=dstloc[:])
            eps_col = cpool.tile([P, 1], f32, tag='eps')
            nc.vector.memset(eps_col[:], LN_EPS)

            # ---- phase A: T1 table (compact src transform) ----
            ACH = 16
            do_A = 'A' in PHASES
            do_B = 'B' in PHASES
            do_D = 'D' in PHASES                       # tiles per load chunk
            for j0 in range(0, E_TBL // P, ACH) if do_A else []:
                jn = min(ACH, E_TBL // P - j0)
                nchunk = wk.tile([97, ACH * P], bf16, tag='nfa')
                nc.gpsimd.dma_start(
                    out=nchunk[:, :jn * P],
                    in_=nftc[:, j0 * P:(j0 + jn) * P])
                for k in range(jn):
                    mm = pp.tile([P, 192], f32, space='PSUM', tag='mm')
                    nc.tensor.matmul(
                        out=mm[:], lhsT=nchunk[:, k * P:(k + 1) * P],
                        rhs=w1_sb[:], start=True, stop=True)
                    t1sb = wk.tile([P, 192], f32, tag='t1sb')
                    nc.any.tensor_copy(out=t1sb[:], in_=mm[:])
                    nc.sync.dma_start(
                        out=t1c[(j0 + k) * P:(j0 + k + 1) * P, :],
                        in_=t1sb[:])

            # ---- phase A2: T2x table (own blocks) ----
            for j0 in range(0, S, ACH) if do_A else []:
                jn = min(ACH, S - j0)
                nchunk = wk.tile([97, ACH * P], bf16, tag='nfa')
                nc.gpsimd.dma_start(
                    out=nchunk[:, :jn * P],
                    in_=nfbT[:, j0 * P:(j0 + jn) * P])
                for k in range(jn):
                    mm = pp.tile([P, 192], f32, space='PSUM', tag='mm')
                    nc.tensor.matmul(
                        out=mm[:], lhsT=nchunk[:, k * P:(k + 1) * P],
                        rhs=w2_sb[:], start=True, stop=True)
                    t2sb = wk.tile([P, 192], bf16, tag='t2sb')
                    nc.any.tensor_copy(out=t2sb[:], in_=mm[:])
                    nc.sync.dma_start(
                        out=t2x[(j0 + k) * P:(j0 + k + 1) * P, :],
                        in_=t2sb[:])

            # ---- phase B + D ----
            slot_first = {s: True for s in range(S)}
            slot_last_tile = {}
            for (s, w, t, off) in sched:
                slot_last_tile[s] = off + t - 1

            cur_slot = -1
            t2blk = None
            sums = None
            for (s, w, t, off) in (sched if do_B else []):
                if s != cur_slot:
                    cur_slot = s
                    t2blk = iop.tile([P, 192], bf16, tag='t2blk')
                    nc.sync.dma_start(
                        out=t2blk[:], in_=t2x[s * P:(s + 1) * P, :])
                    sums = pps.tile([P, 192], f32, space='PSUM', tag='sums')
                # gather this group's T1 rows
                wrows = min(WSZ, E_TBL - w * WSZ)
                gbuf = wk.tile([P, t * 192], f32, tag='gbuf')
                if DO_GATHER:
                    nc.gpsimd.dma_gather(
                        out_ap=gbuf[:].rearrange('p (t d) -> p t d', t=t),
                        in_ap=t1c[w * WSZ:w * WSZ + wrows, :],
                        idxs_ap=idx_all[:, off * 8:(off + t) * 8],
                        num_idxs=t * P,
                        num_idxs_reg=t * P,
                        elem_size=192,
                        single_packet=(t * P <= 512),
                    )
                eftg = iop.tile([H, t * P], bf16, tag='eftg')
                if DO_LOADS:
                    nc.gpsimd.dma_start(
                        out=eftg[:], in_=efT[:, off * P:(off + t) * P])
                for k in range(t):
                    tt = off + k
                    if BLEVEL < 2:
                        continue
                    onehot = wk.tile([P, P], bf16, tag='onehot')
                    nc.vector.tensor_tensor(
                        out=onehot[:],
                        in0=dl_all[:, tt:tt + 1].to_broadcast([P, P]),
                        in1=iota_sb[:], op=ALU.is_equal)
                    ohps = pp.tile([P, P], bf16, space='PSUM', tag='tr')
                    nc.tensor.transpose(
                        out=ohps[:], in_=onehot[:], identity=id_bf[:])
                    ohne = wk.tile([P, P], bf16, tag='ohne')
                    nc.any.tensor_copy(out=ohne[:], in_=ohps[:])

                    if BLEVEL < 3:
                        continue
                    mp = pp.tile([P, 192], f32, space='PSUM', tag='mm')
                    nc.tensor.matmul(
                        out=mp[:, 0:H], lhsT=eftg[:, k * P:(k + 1) * P],
                        rhs=weg_sb[:], start=True, stop=False)
                    nc.tensor.matmul(
                        out=mp[:, 0:H], lhsT=ohne[:], rhs=t2blk[:, 0:H],
                        start=False, stop=True)
                    msb = wk.tile([P, H], f32, tag='msb')
                    nc.vector.tensor_add(
                        out=msb[:], in0=mp[:, 0:H],
                        in1=gbuf[:, k * 192:k * 192 + H])

                    valcat = wk.tile([P, 192], bf16, tag='valcat')
                    nc.scalar.activation(
                        out=valcat[:, 0:H], in_=msb[:], func=AF.Sigmoid)
                    nc.vector.tensor_tensor(
                        out=valcat[:, H:192],
                        in0=gbuf[:, k * 192 + H:(k + 1) * 192],
                        in1=valcat[:, 0:H], op=ALU.mult)

                    if BLEVEL < 4:
                        continue
                    nc.tensor.matmul(
                        out=sums[:], lhsT=onehot[:], rhs=valcat[:],
                        start=slot_first[s],
                        stop=(tt == slot_last_tile[s]))
                    slot_first[s] = False

                    if BLEVEL < 5:
                        continue
                    # layer norm + silu on m
                    bnst = wk.tile([P, 6], f32, tag='bnst')
                    nc.vector.bn_stats(out=bnst[:], in_=msb[:])
                    bnag = wk.tile([P, 2], f32, tag='bnag')
                    nc.vector.bn_aggr(out=bnag[:], in_=bnst[:])
                    std = wk.tile([P, 1], f32, tag='std')
                    nc.scalar.activation(
                        out=std[:], in_=bnag[:, 1:2], func=AF.Sqrt,
                        bias=eps_col[:])
                    rstd = wk.tile([P, 1], f32, tag='rstd')
                    nc.vector.reciprocal(out=rstd[:], in_=std[:])
                    nmr = wk.tile([P, 1], f32, tag='nmr')
                    nc.vector.tensor_scalar(
                        out=nmr[:], in0=bnag[:, 0:1], scalar1=rstd[:],
                        scalar2=-1.0, op0=ALU.mult, op1=ALU.mult)
                    ssb = wk.tile([P, H], bf16, tag='ssb')
                    nc.scalar.activation(
                        out=ssb[:], in_=msb[:], func=AF.Silu,
                        bias=nmr[:], scale=rstd[:])

                    if BLEVEL < 6:
                        continue
                    # y = efT + silu(LN(m)).T
                    yps = pp.tile([P, P], bf16, space='PSUM', tag='tr')
                    nc.tensor.transpose(
                        out=yps[0:H, :], in_=ssb[:], identity=id_bf[:])
                    ysb = wk.tile([H, P], f32, tag='ysb')
                    nc.vector.tensor_add(
                        out=ysb[:], in0=yps[0:H, :],
                        in1=eftg[:, k * P:(k + 1) * P])
                    nc.sync.dma_start(
                        out=yT[:, tt * P:(tt + 1) * P], in_=ysb[:])

                    if tt == slot_last_tile[s] and do_D:
                        # ---- phase D for slot s ----
                        ssd = wk.tile([P, H], f32, tag='ssd')
                        nc.vector.tensor_scalar_add(
                            out=ssd[:], in0=sums[:, 0:H], scalar1=1e-6)
                        rec = wk.tile([P, H], f32, tag='rec')
                        nc.vector.reciprocal(out=rec[:], in_=ssd[:])
                        h = wk.tile([P, H], f32, tag='h')
                        nc.vector.tensor_mul(
                            out=h[:], in0=sums[:, H:192], in1=rec[:])
                        xpre = wk.tile([P, H], f32, tag='xpre')
                        nc.vector.tensor_add(
                            out=xpre[:], in0=h[:], in1=t2blk[:, H:192])
                        bnst2 = wk.tile([P, 6], f32, tag='bnst')
                        nc.vector.bn_stats(out=bnst2[:], in_=xpre[:])
                        bnag2 = wk.tile([P, 2], f32, tag='bnag')
                        nc.vector.bn_aggr(out=bnag2[:], in_=bnst2[:])
                        std2 = wk.tile([P, 1], f32, tag='std')
                        nc.scalar.activation(
                            out=std2[:], in_=bnag2[:, 1:2], func=AF.Sqrt,
                            bias=eps_col[:])
                        rstd2 = wk.tile([P, 1], f32, tag='rstd')
                        nc.vector.reciprocal(out=rstd2[:], in_=std2[:])
                        nmr2 = wk.tile([P, 1], f32, tag='nmr')
                        nc.vector.tensor_scalar(
                            out=nmr2[:], in0=bnag2[:, 0:1], scalar1=rstd2[:],
                            scalar2=-1.0, op0=ALU.mult, op1=ALU.mult)
                        xs = wk.tile([P, H], f32, tag='xs')
                        nc.scalar.activation(
                            out=xs[:], in_=xpre[:], func=AF.Silu,
                            bias=nmr2[:], scale=rstd2[:])
                        nfblk = iop.tile([P, H], f32, tag='nfblk')
                        nc.sync.dma_start(
                            out=nfblk[:], in_=nfb[s * P:(s + 1) * P, :])
                        xres = wk.tile([P, H], f32, tag='xres')
                        nc.vector.tensor_add(
                            out=xres[:], in0=xs[:], in1=nfblk[:])
                        nc.sync.dma_start(
                            out=xout[s * P:(s + 1) * P, :], in_=xres[:])

    nc.finalize()
    return nc


# ----------------------------------------------------------------------------
# top-level
# ----------------------------------------------------------------------------

_TRACE = [False]


def kernel(**inputs):
    from concourse.bass_utils import run_bass_kernel_spmd

    src = np.asarray(inputs['src'])
    dst = np.asarray(inputs['dst'])
    node_feats = np.asarray(inputs['node_feats'], np.float32)
    edge_feats = np.asarray(inputs['edge_feats'], np.float32)
    N, E = node_feats.shape[0], edge_feats.shape[0]

    plan = build_plan(src, dst, N)
    in_maps = build_inputs(plan, inputs)
    nc = build_kernel(plan)
    res = run_bass_kernel_spmd(
        nc, in_maps, core_ids=list(range(NCORES)), trace=_TRACE[0])
    kernel.last_result = res

    x = np.zeros((N, H), np.float32)
    y = np.zeros((E, H), np.float32)
    for c in range(NCORES):
        out = res.results[c]
        blocks = plan['slot_block'][c]
        xs = out['xout'].reshape(plan['S'], P, H)
        for s_i, b in enumerate(blocks):
            lo = b * P
            hi = min(lo + P, N)
            if lo < N:
                x[lo:hi] = xs[s_i, :hi - lo]
        canon = plan['canon_edge'][c]
        real = canon >= 0
        y[canon[real]] = out['yT'][:, real].T
    return x, y


# revision 12
# speedup vs baseline: 2.0067x; 2.0067x over previous
"""ALIGNN edge-gated message passing on 8 Trainium2 NeuronCores.

Strategy: edges partitioned by dst-block across cores (no collectives).
Each core receives host-prepared, per-core data:
  - a compacted node table (only nodes referenced as src by its edges)
  - its own 98 node-blocks (128 nodes each) in load-balanced "slot" order
  - its edge slice in canonical (slot, window, tile) order
Device work per core:
  phase A : PE matmuls node_feats -> T1=[e_src+bias | Bh] (f32, compact rows)
            and T2x=[e_dst | x_lin] (bf16, own blocks)
  phase B : per 128-edge tile: dma_gather T1 rows by src (int16 windows),
            one-hot(dst_local) via is_equal, m = ef@W_eg + onehot.T@T2blk
            (+ gathered, DVE), sigma = sigmoid(m), segment-sum via one-hot
            matmul accumulation in PSUM, y = ef + silu(LN(m)) via PE
            transpose + DVE add
  phase D : per slot: h = ssh/(ss+1e-6); x = nf + silu(LN(x_lin + h))
"""
import sys

if '/opt/trn_rl_repo' not in sys.path:
    sys.path.insert(0, '/opt/trn_rl_repo')

import numpy as np

H = 96
LN_EPS = 1e-5
NCORES = 8
P = 128
WMAX = 32640          # gather window rows (<= int16 max, mult of 128)
PHASES = 'ABD'        # debug bisect: which phases to build
BLEVEL = 9            # debug bisect: phase-B sub-level
DO_GATHER = True
DO_LOADS = True


# ----------------------------------------------------------------------------
# host-side plan
# ----------------------------------------------------------------------------

def build_plan(src, dst, N):
    E = src.shape[0]
    n_blocks_real = (N + P - 1) // P
    n_blocks = ((n_blocks_real + NCORES - 1) // NCORES) * NCORES
    S = n_blocks // NCORES              # slots per core
    N_pad = n_blocks * P

    blk_of_edge = dst // P
    blk_counts = np.bincount(blk_of_edge, minlength=n_blocks)

    # balanced assignment: sort blocks by count desc, greedily fill cores
    order = np.argsort(-blk_counts, kind='stable')
    core_load = np.zeros(NCORES, dtype=np.int64)
    core_nblk = np.zeros(NCORES, dtype=np.int64)
    blk_core = np.zeros(n_blocks, dtype=np.int64)
    for b in order:
        cands = np.where(core_nblk < S)[0]
        c = cands[np.argmin(core_load[cands])]
        blk_core[b] = c
        core_load[c] += blk_counts[b]
        core_nblk[c] += 1

    # per-core slot order: blocks sorted by count desc
    slot_block = np.zeros((NCORES, S), dtype=np.int64)   # slot -> block id
    for c in range(NCORES):
        blks = np.where(blk_core == c)[0]
        blks = blks[np.argsort(-blk_counts[blks], kind='stable')]
        slot_block[c] = blks

    # per-core edges grouped by slot
    edge_core = blk_core[blk_of_edge]
    slot_of_block = np.zeros(n_blocks, dtype=np.int64)
    for c in range(NCORES):
        slot_of_block[slot_block[c]] = np.arange(S)
    edge_slot = slot_of_block[blk_of_edge]

    # compacted src table per core
    srclist = []                        # core -> sorted unique srcs
    for c in range(NCORES):
        u = np.unique(src[edge_core == c])
        srclist.append(u)
    E_TBL = ((max(len(u) for u in srclist) + P - 1) // P) * P
    n_win = max(1, (E_TBL + WMAX - 1) // WMAX)
    WSZ = ((E_TBL // n_win + P - 1) // P) * P
    assert WSZ <= 32767

    # src position within the compact table, per core
    src_pos = np.zeros((NCORES, E), dtype=np.int64)   # only valid on own rows
    for c in range(NCORES):
        m = edge_core == c
        src_pos[c, m] = np.searchsorted(srclist[c], src[m])

    # group = (slot, window); count per (core, slot, window)
    cnt = np.zeros((NCORES, S, n_win), dtype=np.int64)
    for c in range(NCORES):
        m = edge_core == c
        w = src_pos[c, m] // WSZ
        np.add.at(cnt[c], (edge_slot[m], w), 1)
    tiles_sw = np.maximum(np.ceil(cnt / P).astype(np.int64).max(axis=0), 0)
    tiles_sw[:, 0] = np.maximum(tiles_sw[:, 0], 1)   # every slot >=1 tile
    TT = int(tiles_sw.sum())
    E_pad = TT * P

    # schedule: list of (slot, window, ntiles, tile_offset)
    sched = []
    off = 0
    for s in range(S):
        for w in range(n_win):
            t = int(tiles_sw[s, w])
            if t:
                sched.append((s, w, t, off))
                off += t

    # canonical per-core edge placement
    # pos_in_stream for edge e on its core; dummies fill the rest
    canon_edge = np.full((NCORES, E_pad), -1, dtype=np.int64)
    for c in range(NCORES):
        m = np.where(edge_core == c)[0]
        w = src_pos[c, m] // WSZ
        key = edge_slot[m] * n_win + w
        ordr = np.argsort(key, kind='stable')
        me, ke = m[ordr], key[ordr]
        # group boundaries in sorted list
        group_off = {(s_, w_): o_ * P for (s_, w_, t_, o_) in sched}
        pos = np.zeros(len(me), dtype=np.int64)
        start = 0
        for k in np.unique(ke):
            cnt_k = int((ke == k).sum())
            s_, w_ = divmod(int(k), n_win)
            base = group_off[(s_, w_)]
            pos[start:start + cnt_k] = base + np.arange(cnt_k)
            start += cnt_k
        canon_edge[c, pos] = me
    return dict(
        N_pad=N_pad, n_blocks=n_blocks, S=S, E_TBL=E_TBL, n_win=n_win,
        WSZ=WSZ, TT=TT, E_pad=E_pad, sched=sched, slot_block=slot_block,
        srclist=srclist, src_pos=src_pos, canon_edge=canon_edge,
    )


def build_inputs(plan, inputs):
    node_feats = np.asarray(inputs['node_feats'], np.float32)
    edge_feats = np.asarray(inputs['edge_feats'], np.float32)
    src = np.asarray(inputs['src'])
    dst = np.asarray(inputs['dst'])
    N = node_feats.shape[0]

    tp = (np.asarray(inputs['time_feats'], np.float32) @
          np.asarray(inputs['W_tp'], np.float32) +
          np.asarray(inputs['b_tp'], np.float32))[0]
    bias_src = np.asarray(inputs['b_sg'], np.float32) + tp + \
        np.asarray(inputs['b_eg'], np.float32)

    W1b = np.concatenate([
        np.concatenate([inputs['W_sg'], inputs['W_du']], axis=1),
        np.concatenate([bias_src, inputs['b_du']])[None, :],
    ], axis=0).astype(np.float32)                      # [97, 192]
    W2b = np.concatenate([
        np.concatenate([inputs['W_dg'], inputs['W_su']], axis=1),
        np.concatenate([inputs['b_dg'], inputs['b_su']])[None, :],
    ], axis=0).astype(np.float32)                      # [97, 192]

    S, E_TBL, E_pad, TT = plan['S'], plan['E_TBL'], plan['E_pad'], plan['TT']
    nf_pad = np.zeros((plan['N_pad'], H), np.float32)
    nf_pad[:N] = node_feats

    iota = np.tile(np.arange(P, dtype=np.float32), (P, 1))
    ident = np.eye(P, dtype=np.float32)

    in_maps = []
    for c in range(NCORES):
        u = plan['srclist'][c]
        nftc = np.zeros((97, E_TBL), np.float32)
        nftc[:H, :len(u)] = node_feats[u].T
        nftc[96, :] = 1.0

        blocks = plan['slot_block'][c]
        own = nf_pad.reshape(-1, P, H)[blocks]          # [S, 128, 96]
        own_flat = own.reshape(S * P, H)
        nfbT = np.zeros((97, S * P), np.float32)
        nfbT[:H] = own_flat.T
        nfbT[96] = 1.0

        canon = plan['canon_edge'][c]
        real = canon >= 0
        efT = np.zeros((H, E_pad), np.float32)
        efT[:, real] = edge_feats[canon[real]].T

        dstloc = np.full(E_pad, -1.0, np.float32)
        dstloc[real] = (dst[canon[real]] % P).astype(np.float32)
        dstloc = dstloc.reshape(TT, P).T.copy()         # [128, TT]

        gpos = np.zeros(E_pad, np.int64)
        gpos[real] = plan['src_pos'][c, canon[real]] % plan['WSZ']
        gidx = np.zeros((16, E_pad // 16), np.int16)
        idx_lin = np.arange(E_pad)
        gidx[idx_lin % 16, idx_lin // 16] = gpos.astype(np.int16)
        gidx = np.tile(gidx, (8, 1))                    # [128, E_pad/16]

        in_maps.append({
            'nftc': nftc.astype(np.float32),
            'nfbT': nfbT.astype(np.float32),
            'w1b': W1b, 'w2b': W2b,
            'weg': np.asarray(inputs['W_eg'], np.float32),
            'efT': efT,
            'dstloc': dstloc,
            'gidx': gidx,
            'iota': iota,
            'ident': ident,
            'nfb': own_flat,
        })
    return in_maps


# ----------------------------------------------------------------------------
# device kernel
# ----------------------------------------------------------------------------

def build_kernel(plan):
    import concourse.bacc as bacc
    import concourse.bass as bass
    import concourse.mybir as mybir
    import concourse.tile as tile

    f32, bf16, i16 = mybir.dt.float32, mybir.dt.bfloat16, mybir.dt.int16
    AF = mybir.ActivationFunctionType
    ALU = mybir.AluOpType

    S, E_TBL, E_pad, TT = plan['S'], plan['E_TBL'], plan['E_pad'], plan['TT']
    n_win, WSZ = plan['n_win'], plan['WSZ']
    sched = plan['sched']
    NB = S * P                                       # own nodes per core
    DG = 32                                          # deferred-LN group size

    nc = bacc.Bacc()
    dp = nc.declare_dram_parameter
    nftc = dp('nftc', [97, E_TBL], f32, isOutput=False)
    nfbT = dp('nfbT', [97, NB], f32, isOutput=False)
    w1b = dp('w1b', [97, 192], f32, isOutput=False)
    w2b = dp('w2b', [97, 192], f32, isOutput=False)
    weg = dp('weg', [H, H], f32, isOutput=False)
    efT = dp('efT', [H, E_pad], f32, isOutput=False)
    dstloc = dp('dstloc', [P, TT], f32, isOutput=False)
    gidx = dp('gidx', [P, E_pad // 16], i16, isOutput=False)
    iota = dp('iota', [P, P], f32, isOutput=False)
    ident = dp('ident', [P, P], f32, isOutput=False)
    nfb = dp('nfb', [NB, H], f32, isOutput=False)
    yT = dp('yT', [H, E_pad], f32, isOutput=True)
    xout = dp('xout', [NB, H], f32, isOutput=True)

    t1c = nc.dram_tensor('t1c', [E_TBL, 192], f32)
    t2x = nc.dram_tensor('t2x', [NB, 192], bf16)

    with tile.TileContext(nc) as tc:
        with (
            tc.tile_pool(name='const', bufs=1) as cpool,
            tc.tile_pool(name='io', bufs=3) as iop,
            tc.tile_pool(name='eft', bufs=12) as efp,
            tc.tile_pool(name='msb', bufs=DG + 6) as msp,
            tc.tile_pool(name='work', bufs=3) as wk,
            tc.tile_pool(name='grp', bufs=2) as grp,
            tc.tile_pool(name='ps', bufs=2, space='PSUM') as pp,
            tc.tile_pool(name='ps_sum', bufs=2, space='PSUM') as pps,
        ):
            # ---- constants ----
            iota_sb = cpool.tile([P, P], f32, tag='iota')
            nc.sync.dma_start(out=iota_sb[:], in_=iota[:])
            id_bf = cpool.tile([P, P], bf16, tag='idb')
            nc.gpsimd.dma_start(out=id_bf[:], in_=ident[:])   # cast f32->bf16
            w1_sb = cpool.tile([97, 192], bf16, tag='w1')
            nc.gpsimd.dma_start(out=w1_sb[:], in_=w1b[:])
            w2_sb = cpool.tile([97, 192], bf16, tag='w2')
            nc.gpsimd.dma_start(out=w2_sb[:], in_=w2b[:])
            weg_sb = cpool.tile([H, H], bf16, tag='weg')
            nc.gpsimd.dma_start(out=weg_sb[:], in_=weg[:])
            idx_all = cpool.tile([P, E_pad // 16], i16, tag='gidx')
            nc.sync.dma_start(out=idx_all[:], in_=gidx[:])
            dl_all = cpool.tile([P, TT], f32, tag='dstloc')
            nc.sync.dma_start(out=dl_all[:], in_=dstloc[:])
            eps_col = cpool.tile([P, 1], f32, tag='eps')
            nc.vector.memset(eps_col[:], LN_EPS)
            xpre_all = cpool.tile([P, S * H], f32, tag='xpre')

            # ---- phase A: T1 table (compact src transform) ----
            ACH = 16
            for j0 in range(0, E_TBL // P, ACH):
                jn = min(ACH, E_TBL // P - j0)
                nchunk = wk.tile([97, ACH * P], bf16, tag='nfa')
                nc.gpsimd.dma_start(
                    out=nchunk[:, :jn * P],
                    in_=nftc[:, j0 * P:(j0 + jn) * P])
                for k in range(jn):
                    mm = pp.tile([P, 192], f32, space='PSUM', tag='mm')
                    nc.tensor.matmul(
                        out=mm[:], lhsT=nchunk[:, k * P:(k + 1) * P],
                        rhs=w1_sb[:], start=True, stop=True)
                    t1sb = wk.tile([P, 192], f32, tag='t1sb')
                    nc.vector.tensor_copy(out=t1sb[:], in_=mm[:])
                    nc.sync.dma_start(
                        out=t1c[(j0 + k) * P:(j0 + k + 1) * P, :],
                        in_=t1sb[:])

            # ---- phase A2: T2x table (own blocks) ----
            for j0 in range(0, S, ACH):
                jn = min(ACH, S - j0)
                nchunk = wk.tile([97, ACH * P], bf16, tag='nfa')
                nc.gpsimd.dma_start(
                    out=nchunk[:, :jn * P],
                    in_=nfbT[:, j0 * P:(j0 + jn) * P])
                for k in range(jn):
                    mm = pp.tile([P, 192], f32, space='PSUM', tag='mm')
                    nc.tensor.matmul(
                        out=mm[:], lhsT=nchunk[:, k * P:(k + 1) * P],
                        rhs=w2_sb[:], start=True, stop=True)
                    t2sb = wk.tile([P, 192], bf16, tag='t2sb')
                    nc.vector.tensor_copy(out=t2sb[:], in_=mm[:])
                    nc.sync.dma_start(
                        out=t2x[(j0 + k) * P:(j0 + k + 1) * P, :],
                        in_=t2sb[:])

            # ---- phase B: edges (sigma + scatter inline; LN/silu deferred) --
            slot_first = {s: True for s in range(S)}
            slot_last_tile = {}
            for (s, w, t, off) in sched:
                slot_last_tile[s] = off + t - 1

            pending = []          # [(tt, msb, eftg, k)]
            stats_buf = [None]
            pend_eps = []         # deferred per-slot xpre LN records: (s,)

            def flush():
                if not pending:
                    return
                g = len(pending)
                st = stats_buf[0]
                stv = st[:].rearrange('p (g s) -> p g s', s=6)
                a1 = grp.tile([P, DG], f32, tag='a1')
                nc.vector.tensor_add(
                    out=a1[:, :g], in0=stv[:, :g, 2], in1=stv[:, :g, 5])
                a2 = grp.tile([P, DG], f32, tag='a2')
                nc.vector.tensor_sub(
                    out=a2[:, :g], in0=stv[:, :g, 1], in1=stv[:, :g, 4])
                a3 = grp.tile([P, DG], f32, tag='a3')
                nc.vector.tensor_mul(out=a3[:, :g], in0=a2[:, :g], in1=a2[:, :g])
                var = grp.tile([P, DG], f32, tag='var')
                nc.vector.tensor_scalar(
                    out=var[:, :g], in0=a1[:, :g], scalar1=1.0 / 96.0,
                    scalar2=None, op0=ALU.mult)
                nc.vector.tensor_scalar(
                    out=a3[:, :g], in0=a3[:, :g], scalar1=0.25,
                    scalar2=None, op0=ALU.mult)
                nc.vector.tensor_add(out=var[:, :g], in0=var[:, :g], in1=a3[:, :g])
                std = grp.tile([P, DG], f32, tag='std')
                nc.scalar.activation(
                    out=std[:, :g], in_=var[:, :g], func=AF.Sqrt, bias=eps_col[:])
                rstd = grp.tile([P, DG], f32, tag='rstd')
                nc.vector.reciprocal(out=rstd[:, :g], in_=std[:, :g])
                msum = grp.tile([P, DG], f32, tag='msum')
                nc.vector.tensor_add(
                    out=msum[:, :g], in0=stv[:, :g, 1], in1=stv[:, :g, 4])
                nmr = grp.tile([P, DG], f32, tag='nmr')
                nc.vector.tensor_mul(out=nmr[:, :g], in0=msum[:, :g], in1=rstd[:, :g])
                nc.vector.tensor_scalar(
                    out=nmr[:, :g], in0=nmr[:, :g], scalar1=-0.5,
                    scalar2=None, op0=ALU.mult)
                for (tt, msb, eftg, kk) in pending:
                    j = pending.index((tt, msb, eftg, kk))
                    ssb = wk.tile([P, H], bf16, tag='ssb')
                    nc.scalar.activation(
                        out=ssb[:], in_=msb[:], func=AF.Silu,
                        bias=nmr[:, j:j + 1], scale=rstd[:, j:j + 1])
                    yps = pp.tile([P, P], bf16, space='PSUM', tag='tr')
                    nc.tensor.transpose(
                        out=yps[0:H, :], in_=ssb[:], identity=id_bf[:])
                    ysb = wk.tile([H, P], f32, tag='ysb')
                    nc.vector.tensor_add(
                        out=ysb[:], in0=yps[0:H, :],
                        in1=eftg[:, kk * P:(kk + 1) * P])
                    nc.sync.dma_start(
                        out=yT[:, tt * P:(tt + 1) * P], in_=ysb[:])
                pending.clear()
                stats_buf[0] = None

            cur_slot = -1
            t2blk = None
            sums = None
            for (s, w, t, off) in sched:
                if s != cur_slot:
                    cur_slot = s
                    t2blk = iop.tile([P, 192], bf16, tag='t2blk')
                    nc.sync.dma_start(
                        out=t2blk[:], in_=t2x[s * P:(s + 1) * P, :])
                    sums = pps.tile([P, 192], f32, space='PSUM', tag='sums')
                wrows = min(WSZ, E_TBL - w * WSZ)
                gbuf = wk.tile([P, t * 192], f32, tag='gbuf')
                nc.gpsimd.dma_gather(
                    out_ap=gbuf[:].rearrange('p (t d) -> p t d', t=t),
                    in_ap=t1c[w * WSZ:w * WSZ + wrows, :],
                    idxs_ap=idx_all[:, off * 8:(off + t) * 8],
                    num_idxs=t * P,
                    num_idxs_reg=t * P,
                    elem_size=192,
                    single_packet=(t * P <= 512),
                )
                eftg = efp.tile([H, t * P], bf16, tag='eftg')
                nc.gpsimd.dma_start(
                    out=eftg[:], in_=efT[:, off * P:(off + t) * P])
                for k in range(t):
                    tt = off + k
                    onehot = wk.tile([P, P], bf16, tag='onehot')
                    nc.vector.tensor_tensor(
                        out=onehot[:],
                        in0=dl_all[:, tt:tt + 1].to_broadcast([P, P]),
                        in1=iota_sb[:], op=ALU.is_equal)
                    ohps = pp.tile([P, P], bf16, space='PSUM', tag='tr')
                    nc.tensor.transpose(
                        out=ohps[:], in_=onehot[:], identity=id_bf[:])
                    ohne = wk.tile([P, P], bf16, tag='ohne')
                    nc.vector.tensor_copy(out=ohne[:], in_=ohps[:])

                    mp = pp.tile([P, 192], f32, space='PSUM', tag='mm')
                    nc.tensor.matmul(
                        out=mp[:, 0:H], lhsT=eftg[:, k * P:(k + 1) * P],
                        rhs=weg_sb[:], start=True, stop=False)
                    nc.tensor.matmul(
                        out=mp[:, 0:H], lhsT=ohne[:], rhs=t2blk[:, 0:H],
                        start=False, stop=True)
                    msb = msp.tile([P, H], f32, tag='msb')
                    nc.vector.tensor_add(
                        out=msb[:], in0=mp[:, 0:H],
                        in1=gbuf[:, k * 192:k * 192 + H])

                    valcat = wk.tile([P, 192], bf16, tag='valcat')
                    nc.scalar.activation(
                        out=valcat[:, 0:H], in_=msb[:], func=AF.Sigmoid)
                    nc.vector.tensor_tensor(
                        out=valcat[:, H:192],
                        in0=gbuf[:, k * 192 + H:(k + 1) * 192],
                        in1=valcat[:, 0:H], op=ALU.mult)

                    nc.tensor.matmul(
                        out=sums[:], lhsT=onehot[:], rhs=valcat[:],
                        start=slot_first[s],
                        stop=(tt == slot_last_tile[s]))
                    slot_first[s] = False

                    if stats_buf[0] is None:
                        st_new = grp.tile([P, DG * 6], f32, tag='stats')
                        stats_buf[0] = st_new
                    j = len(pending)
                    nc.vector.bn_stats(
                        out=stats_buf[0][:, j * 6:(j + 1) * 6], in_=msb[:])
                    pending.append((tt, msb, eftg, k))
                    if len(pending) == DG:
                        flush()

                    if tt == slot_last_tile[s]:
                        # xpre for slot s (LN deferred to final phase)
                        ssd = wk.tile([P, H], f32, tag='ssd')
                        nc.vector.tensor_scalar_add(
                            out=ssd[:], in0=sums[:, 0:H], scalar1=1e-6)
                        rec = wk.tile([P, H], f32, tag='rec')
                        nc.vector.reciprocal(out=rec[:], in_=ssd[:])
                        h = wk.tile([P, H], f32, tag='h')
                        nc.vector.tensor_mul(
                            out=h[:], in0=sums[:, H:192], in1=rec[:])
                        nc.vector.tensor_add(
                            out=xpre_all[:, s * H:(s + 1) * H],
                            in0=h[:], in1=t2blk[:, H:192])
            flush()

            # ---- final phase: node LN + silu + residual ----
            for s0 in range(0, S, DG):
                g = min(DG, S - s0)
                st = grp.tile([P, DG * 6], f32, tag='stats')
                for k in range(g):
                    nc.vector.bn_stats(
                        out=st[:, k * 6:(k + 1) * 6],
                        in_=xpre_all[:, (s0 + k) * H:(s0 + k + 1) * H])
                stv = st[:].rearrange('p (g s) -> p g s', s=6)
                a1 = grp.tile([P, DG], f32, tag='a1')
                nc.vector.tensor_add(
                    out=a1[:, :g], in0=stv[:, :g, 2], in1=stv[:, :g, 5])
                a2 = grp.tile([P, DG], f32, tag='a2')
                nc.vector.tensor_sub(
                    out=a2[:, :g], in0=stv[:, :g, 1], in1=stv[:, :g, 4])
                a3 = grp.tile([P, DG], f32, tag='a3')
                nc.vector.tensor_mul(out=a3[:, :g], in0=a2[:, :g], in1=a2[:, :g])
                var = grp.tile([P, DG], f32, tag='var')
                nc.vector.tensor_scalar(
                    out=var[:, :g], in0=a1[:, :g], scalar1=1.0 / 96.0,
                    scalar2=None, op0=ALU.mult)
                nc.vector.tensor_scalar(
                    out=a3[:, :g], in0=a3[:, :g], scalar1=0.25,
                    scalar2=None, op0=ALU.mult)
                nc.vector.tensor_add(out=var[:, :g], in0=var[:, :g], in1=a3[:, :g])
                std = grp.tile([P, DG], f32, tag='std')
                nc.scalar.activation(
                    out=std[:, :g], in_=var[:, :g], func=AF.Sqrt, bias=eps_col[:])
                rstd = grp.tile([P, DG], f32, tag='rstd')
                nc.vector.reciprocal(out=rstd[:, :g], in_=std[:, :g])
                msum = grp.tile([P, DG], f32, tag='msum')
                nc.vector.tensor_add(
                    out=msum[:, :g], in0=stv[:, :g, 1], in1=stv[:, :g, 4])
                nmr = grp.tile([P, DG], f32, tag='nmr')
                nc.vector.tensor_mul(out=nmr[:, :g], in0=msum[:, :g], in1=rstd[:, :g])
                nc.vector.tensor_scalar(
                    out=nmr[:, :g], in0=nmr[:, :g], scalar1=-0.5,
                    scalar2=None, op0=ALU.mult)
                for k in range(g):
                    s = s0 + k
                    xs = wk.tile([P, H], f32, tag='xs')
                    nc.scalar.activation(
                        out=xs[:], in_=xpre_all[:, s * H:(s + 1) * H],
                        func=AF.Silu, bias=nmr[:, k:k + 1], scale=rstd[:, k:k + 1])
                    nfblk = iop.tile([P, H], f32, tag='nfblk')
                    nc.sync.dma_start(
                        out=nfblk[:], in_=nfb[s * P:(s + 1) * P, :])
                    xres = wk.tile([P, H], f32, tag='xres')
                    nc.vector.tensor_add(out=xres[:], in0=xs[:], in1=nfblk[:])
                    nc.sync.dma_start(
                        out=xout[s * P:(s + 1) * P, :], in_=xres[:])

    nc.finalize()
    return nc


# ----------------------------------------------------------------------------
# top-level
# ----------------------------------------------------------------------------

_TRACE = [False]


def kernel(**inputs):
    from concourse.bass_utils import run_bass_kernel_spmd

    src = np.asarray(inputs['src'])
    dst = np.asarray(inputs['dst'])
    node_feats = np.asarray(inputs['node_feats'], np.float32)
    edge_feats = np.asarray(inputs['edge_feats'], np.float32)
    N, E = node_feats.shape[0], edge_feats.shape[0]

    plan = build_plan(src, dst, N)
    in_maps = build_inputs(plan, inputs)
    nc = build_kernel(plan)
    res = run_bass_kernel_spmd(
        nc, in_maps, core_ids=list(range(NCORES)), trace=_TRACE[0])
    kernel.last_result = res

    x = np.zeros((N, H), np.float32)
    y = np.zeros((E, H), np.float32)
    for c in range(NCORES):
        out = res.results[c]
        blocks = plan['slot_block'][c]
        xs = out['xout'].reshape(plan['S'], P, H)
        for s_i, b in enumerate(blocks):
            lo = b * P
            hi = min(lo + P, N)
            if lo < N:
                x[lo:hi] = xs[s_i, :hi - lo]
        canon = plan['canon_edge'][c]
        real = canon >= 0
        y[canon[real]] = out['yT'][:, real].T
    return x, y


# revision 17
# speedup vs baseline: 3.9547x; 1.9707x over previous
"""ALIGNN edge-gated message passing on 8 Trainium2 NeuronCores.

Strategy: edges partitioned by dst-block across cores (no collectives).
Each core receives host-prepared, per-core data:
  - a compacted node table (only nodes referenced as src by its edges)
  - its own 98 node-blocks (128 nodes each) in load-balanced "slot" order
  - its edge slice in canonical (slot, window, tile) order
Device work per core:
  phase A : PE matmuls node_feats -> T1=[e_src+bias | Bh] (f32, compact rows)
            and T2x=[e_dst | x_lin] (bf16, own blocks)
  phase B : per (slot,window) group of 128-edge tiles: dma_gather T1 rows by
            src (int16 windows), one-hot(dst_local) via batched is_equal,
            m = ef@W_eg + onehot.T@T2blk (+ gathered), sigma = sigmoid(m),
            segment-sum via one-hot matmul accumulation in PSUM;
            LN sqrt batched over deferred groups of 32 tiles, then
            y = ef + silu(LN(m)) written edge-major
  phase D : per slot: xpre = x_lin + ssh/(ss+1e-6); final phase applies
            LN + silu + residual for all slots with batched stats
"""
import sys

if '/opt/trn_rl_repo' not in sys.path:
    sys.path.insert(0, '/opt/trn_rl_repo')

import numpy as np
import ml_dtypes

BF16 = ml_dtypes.bfloat16
H = 96
LN_EPS = 1e-5
NCORES = 8
P = 128
WMAX = 32640          # gather window rows (<= int16 max, mult of 128)
DG = 32               # deferred-LN group size (tiles)


# ----------------------------------------------------------------------------
# host-side plan
# ----------------------------------------------------------------------------

def build_plan(src, dst, N):
    E = src.shape[0]
    n_blocks_real = (N + P - 1) // P
    n_blocks = ((n_blocks_real + NCORES - 1) // NCORES) * NCORES
    S = n_blocks // NCORES              # slots per core
    N_pad = n_blocks * P

    blk_of_edge = dst // P
    blk_counts = np.bincount(blk_of_edge, minlength=n_blocks)

    # balanced assignment: sort blocks by count desc, greedily fill cores
    order = np.argsort(-blk_counts, kind='stable')
    core_load = np.zeros(NCORES, dtype=np.int64)
    core_nblk = np.zeros(NCORES, dtype=np.int64)
    blk_core = np.zeros(n_blocks, dtype=np.int64)
    for b in order:
        cands = np.where(core_nblk < S)[0]
        c = cands[np.argmin(core_load[cands])]
        blk_core[b] = c
        core_load[c] += blk_counts[b]
        core_nblk[c] += 1

    # per-core slot order: blocks sorted by count desc
    slot_block = np.zeros((NCORES, S), dtype=np.int64)   # slot -> block id
    for c in range(NCORES):
        blks = np.where(blk_core == c)[0]
        blks = blks[np.argsort(-blk_counts[blks], kind='stable')]
        slot_block[c] = blks

    edge_core = blk_core[blk_of_edge]
    slot_of_block = np.zeros(n_blocks, dtype=np.int64)
    for c in range(NCORES):
        slot_of_block[slot_block[c]] = np.arange(S)
    edge_slot = slot_of_block[blk_of_edge]

    # compacted src table per core
    srclist = []
    for c in range(NCORES):
        u = np.unique(src[edge_core == c])
        srclist.append(u)
    E_TBL = ((max(len(u) for u in srclist) + P - 1) // P) * P
    n_win = max(1, (E_TBL + WMAX - 1) // WMAX)
    WSZ = ((E_TBL // n_win + P - 1) // P) * P
    assert WSZ <= 32767

    src_pos = np.zeros((NCORES, E), dtype=np.int64)
    for c in range(NCORES):
        m = edge_core == c
        src_pos[c, m] = np.searchsorted(srclist[c], src[m])

    cnt = np.zeros((NCORES, S, n_win), dtype=np.int64)
    for c in range(NCORES):
        m = edge_core == c
        w = src_pos[c, m] // WSZ
        np.add.at(cnt[c], (edge_slot[m], w), 1)
    tiles_sw = np.maximum(np.ceil(cnt / P).astype(np.int64).max(axis=0), 0)
    tiles_sw[:, 0] = np.maximum(tiles_sw[:, 0], 1)   # every slot >=1 tile
    TT = int(tiles_sw.sum())
    E_pad = TT * P

    sched = []
    off = 0
    TMAXG = 5
    for s in range(S):
        for w in range(n_win):
            t = int(tiles_sw[s, w])
            while t > 0:
                tc_ = min(t, TMAXG)
                sched.append((s, w, tc_, off))
                off += tc_
                t -= tc_

    canon_edge = np.full((NCORES, E_pad), -1, dtype=np.int64)
    for c in range(NCORES):
        m = np.where(edge_core == c)[0]
        w = src_pos[c, m] // WSZ
        key = edge_slot[m] * n_win + w
        ordr = np.argsort(key, kind='stable')
        me, ke = m[ordr], key[ordr]
        group_off = {}
        for (s_, w_, t_, o_) in sched:
            if (s_, w_) not in group_off:
                group_off[(s_, w_)] = o_ * P
        pos = np.zeros(len(me), dtype=np.int64)
        start = 0
        for k in np.unique(ke):
            cnt_k = int((ke == k).sum())
            s_, w_ = divmod(int(k), n_win)
            base = group_off[(s_, w_)]
            pos[start:start + cnt_k] = base + np.arange(cnt_k)
            start += cnt_k
        canon_edge[c, pos] = me
    return dict(
        N_pad=N_pad, n_blocks=n_blocks, S=S, E_TBL=E_TBL, n_win=n_win,
        WSZ=WSZ, TT=TT, E_pad=E_pad, sched=sched, slot_block=slot_block,
        srclist=srclist, src_pos=src_pos, canon_edge=canon_edge,
    )


def build_inputs(plan, inputs):
    node_feats = np.asarray(inputs['node_feats'], np.float32)
    edge_feats = np.asarray(inputs['edge_feats'], np.float32)
    src = np.asarray(inputs['src'])
    dst = np.asarray(inputs['dst'])
    N = node_feats.shape[0]

    tp = (np.asarray(inputs['time_feats'], np.float32) @
          np.asarray(inputs['W_tp'], np.float32) +
          np.asarray(inputs['b_tp'], np.float32))[0]
    bias_src = np.asarray(inputs['b_sg'], np.float32) + tp + \
        np.asarray(inputs['b_eg'], np.float32)

    W1b = np.concatenate([
        np.concatenate([inputs['W_sg'], inputs['W_du']], axis=1),
        np.concatenate([bias_src, inputs['b_du']])[None, :],
    ], axis=0).astype(np.float32)                      # [97, 192]
    W2b = np.concatenate([
        np.concatenate([inputs['W_dg'], inputs['W_su']], axis=1),
        np.concatenate([inputs['b_dg'], inputs['b_su']])[None, :],
    ], axis=0).astype(np.float32)                      # [97, 192]

    S, E_TBL, E_pad, TT = plan['S'], plan['E_TBL'], plan['E_pad'], plan['TT']
    nf_pad = np.zeros((plan['N_pad'], H), np.float32)
    nf_pad[:N] = node_feats

    iota = np.tile(np.arange(P, dtype=np.float32), (P, 1))
    ident = np.eye(P, dtype=np.float32)

    in_maps = []
    for c in range(NCORES):
        u = plan['srclist'][c]
        nftc = np.zeros((97, E_TBL), np.float32)
        nftc[:H, :len(u)] = node_feats[u].T
        nftc[96, :] = 1.0

        blocks = plan['slot_block'][c]
        own = nf_pad.reshape(-1, P, H)[blocks]          # [S, 128, 96]
        own_flat = own.reshape(S * P, H)
        nfbT = np.zeros((97, S * P), np.float32)
        nfbT[:H] = own_flat.T
        nfbT[96] = 1.0

        canon = plan['canon_edge'][c]
        real = canon >= 0
        ef_can = np.zeros((E_pad, H), np.float32)
        ef_can[real] = edge_feats[canon[real]]

        dstloc = np.full(E_pad, -1.0, np.float32)
        dstloc[real] = (dst[canon[real]] % P).astype(np.float32)
        dstloc = dstloc.reshape(TT, P).T.copy()         # [128, TT]

        gpos = np.zeros(E_pad, np.int64)
        gpos[real] = plan['src_pos'][c, canon[real]] % plan['WSZ']
        gidx = np.zeros((16, E_pad // 16), np.int16)
        idx_lin = np.arange(E_pad)
        gidx[idx_lin % 16, idx_lin // 16] = gpos.astype(np.int16)
        gidx = np.tile(gidx, (8, 1))                    # [128, E_pad/16]

        in_maps.append({
            'nftc': nftc.astype(BF16),
            'nfbT': nfbT.astype(BF16),
            'w1b': W1b.astype(BF16), 'w2b': W2b.astype(BF16),
            'weg': np.asarray(inputs['W_eg'], np.float32).astype(BF16),
            'efT': ef_can.T.astype(BF16).copy(),
            'ef_em': ef_can.astype(BF16),
            'dstloc': dstloc,
            'gidx': gidx,
            'iota': iota,
            'ident': ident.astype(BF16),
            'nfb': own_flat,
        })
    return in_maps


# ----------------------------------------------------------------------------
# device kernel
# ----------------------------------------------------------------------------

def build_kernel(plan):
    import concourse.bacc as bacc
    import concourse.bass as bass
    import concourse.mybir as mybir
    import concourse.tile as tile

    f32, bf16, i16 = mybir.dt.float32, mybir.dt.bfloat16, mybir.dt.int16
    AF = mybir.ActivationFunctionType
    ALU = mybir.AluOpType

    S, E_TBL, E_pad, TT = plan['S'], plan['E_TBL'], plan['E_pad'], plan['TT']
    n_win, WSZ = plan['n_win'], plan['WSZ']
    sched = plan['sched']
    NB = S * P

    nc = bacc.Bacc()
    dp = nc.declare_dram_parameter
    nftc = dp('nftc', [97, E_TBL], bf16, isOutput=False)
    nfbT = dp('nfbT', [97, NB], bf16, isOutput=False)
    w1b = dp('w1b', [97, 192], bf16, isOutput=False)
    w2b = dp('w2b', [97, 192], bf16, isOutput=False)
    weg = dp('weg', [H, H], bf16, isOutput=False)
    efT = dp('efT', [H, E_pad], bf16, isOutput=False)
    ef_em = dp('ef_em', [E_pad, H], bf16, isOutput=False)
    dstloc = dp('dstloc', [P, TT], f32, isOutput=False)
    gidx = dp('gidx', [P, E_pad // 16], i16, isOutput=False)
    iota = dp('iota', [P, P], f32, isOutput=False)
    ident = dp('ident', [P, P], bf16, isOutput=False)
    nfb = dp('nfb', [NB, H], f32, isOutput=False)
    y_em = dp('y_em', [E_pad, H], f32, isOutput=True)
    xout = dp('xout', [NB, H], f32, isOutput=True)

    t1c = nc.dram_tensor('t1c', [E_TBL, 192], f32)
    t2x = nc.dram_tensor('t2x', [NB, 192], bf16)

    with tile.TileContext(nc) as tc:
        with (
            tc.tile_pool(name='const', bufs=1) as cpool,
            tc.tile_pool(name='io', bufs=4) as iop,
            tc.tile_pool(name='pa', bufs=2) as pa,
            tc.tile_pool(name='eft', bufs=10) as efp,
            tc.tile_pool(name='msb', bufs=10) as msp,
            tc.tile_pool(name='work', bufs=4) as wk,
            tc.tile_pool(name='grp', bufs=2) as grp,
            tc.tile_pool(name='yb', bufs=2) as ybp,
            tc.tile_pool(name='ps', bufs=3, space='PSUM') as pp,
            tc.tile_pool(name='ps_sum', bufs=2, space='PSUM') as pps,
        ):
            # ---- constants ----
            iota_sb = cpool.tile([P, P], f32, tag='iota')
            nc.sync.dma_start(out=iota_sb[:], in_=iota[:])
            id_bf = cpool.tile([P, P], bf16, tag='idb')
            nc.sync.dma_start(out=id_bf[:], in_=ident[:])
            w1_sb = cpool.tile([97, 192], bf16, tag='w1')
            nc.sync.dma_start(out=w1_sb[:], in_=w1b[:])
            w2_sb = cpool.tile([97, 192], bf16, tag='w2')
            nc.sync.dma_start(out=w2_sb[:], in_=w2b[:])
            weg_sb = cpool.tile([H, H], bf16, tag='weg')
            nc.sync.dma_start(out=weg_sb[:], in_=weg[:])
            idx_all = cpool.tile([P, E_pad // 16], i16, tag='gidx')
            nc.sync.dma_start(out=idx_all[:], in_=gidx[:])
            dl_all = cpool.tile([P, TT], f32, tag='dstloc')
            nc.sync.dma_start(out=dl_all[:], in_=dstloc[:])
            eps_col = cpool.tile([P, 1], f32, tag='eps')
            nc.vector.memset(eps_col[:], LN_EPS)
            xpre_all = cpool.tile([P, S * H], f32, tag='xpre')

            # ---- phase A: node transform tables ----
            ACH = 8
            for (tbl, srcT, wsb, n_tiles, odt) in (
                (t1c, nftc, w1_sb, E_TBL // P, f32),
                (t2x, nfbT, w2_sb, S, bf16),
            ):
                for j0 in range(0, n_tiles, ACH):
                    jn = min(ACH, n_tiles - j0)
                    nchunk = pa.tile([97, ACH * P], bf16, tag='nfa')
                    nc.sync.dma_start(
                        out=nchunk[:, :jn * P],
                        in_=srcT[:, j0 * P:(j0 + jn) * P])
                    tbuf = pa.tile([P, ACH * 192], odt, tag='tbuf')
                    for k in range(jn):
                        mm = pp.tile([P, 192], f32, space='PSUM', tag='mm')
                        nc.tensor.matmul(
                            out=mm[:], lhsT=nchunk[:, k * P:(k + 1) * P],
                            rhs=wsb[:], start=True, stop=True)
                        nc.vector.tensor_copy(
                            out=tbuf[:, k * 192:(k + 1) * 192], in_=mm[:])
                    nc.sync.dma_start(
                        out=tbl[j0 * P:(j0 + jn) * P, :].rearrange(
                            '(j p) d -> p j d', p=P),
                        in_=tbuf[:, :jn * 192].rearrange(
                            'p (j d) -> p j d', d=192))

            # ---- phase B ----
            slot_first = {s: True for s in range(S)}
            slot_last = {}
            for (s, w, t, off) in sched:
                slot_last[s] = off + t - 1

            pending = []     # (off, t, msb, efg) per (s,w) group
            pend_n = [0]
            stats_buf = [None]

            def ln_coeffs(st, g):
                """Batched LN: stats [P, g, 6] -> (rstd, nmr) [P, g]."""
                stv = st[:].rearrange('p (g s) -> p g s', s=6)
                a1 = grp.tile([P, DG], f32, tag='a1')
                nc.vector.tensor_add(
                    out=a1[:, :g], in0=stv[:, :g, 2], in1=stv[:, :g, 5])
                a2 = grp.tile([P, DG], f32, tag='a2')
                nc.vector.tensor_sub(
                    out=a2[:, :g], in0=stv[:, :g, 1], in1=stv[:, :g, 4])
                a3 = grp.tile([P, DG], f32, tag='a3')
                nc.vector.tensor_mul(
                    out=a3[:, :g], in0=a2[:, :g], in1=a2[:, :g])
                var = grp.tile([P, DG], f32, tag='var')
                nc.vector.tensor_scalar(
                    out=var[:, :g], in0=a1[:, :g], scalar1=1.0 / 96.0,
                    scalar2=None, op0=ALU.mult)
                nc.vector.tensor_scalar(
                    out=a3[:, :g], in0=a3[:, :g], scalar1=0.25,
                    scalar2=None, op0=ALU.mult)
                nc.vector.tensor_add(
                    out=var[:, :g], in0=var[:, :g], in1=a3[:, :g])
                std = grp.tile([P, DG], f32, tag='std')
                nc.scalar.activation(
                    out=std[:, :g], in_=var[:, :g], func=AF.Sqrt,
                    bias=eps_col[:])
                rstd = grp.tile([P, DG], f32, tag='rstd')
                nc.vector.reciprocal(out=rstd[:, :g], in_=std[:, :g])
                msum = grp.tile([P, DG], f32, tag='msum')
                nc.vector.tensor_add(
                    out=msum[:, :g], in0=stv[:, :g, 1], in1=stv[:, :g, 4])
                nmr = grp.tile([P, DG], f32, tag='nmr')
                nc.vector.tensor_mul(
                    out=nmr[:, :g], in0=msum[:, :g], in1=rstd[:, :g])
                nc.vector.tensor_scalar(
                    out=nmr[:, :g], in0=nmr[:, :g], scalar1=-0.5,
                    scalar2=None, op0=ALU.mult)
                return rstd, nmr

            def flush():
                if not pending:
                    return
                g = pend_n[0]
                rstd, nmr = ln_coeffs(stats_buf[0], g)
                ybuf = ybp.tile([P, DG * H], f32, tag='ybuf')
                j = 0
                off0 = pending[0][0]
                for (off_, t_, msb_, efg_) in pending:
                    for k in range(t_):
                        nc.scalar.activation(
                            out=ybuf[:, (j + k) * H:(j + k + 1) * H],
                            in_=msb_[:, k * H:(k + 1) * H],
                            func=AF.Silu,
                            bias=nmr[:, j + k:j + k + 1],
                            scale=rstd[:, j + k:j + k + 1])
                    nc.vector.tensor_add(
                        out=ybuf[:, j * H:(j + t_) * H],
                        in0=ybuf[:, j * H:(j + t_) * H],
                        in1=efg_[:])
                    j += t_
                nc.sync.dma_start(
                    out=y_em[off0 * P:(off0 + g) * P, :].rearrange(
                        '(j p) f -> p j f', p=P),
                    in_=ybuf[:, :g * H].rearrange('p (j f) -> p j f', f=H))
                pending.clear()
                pend_n[0] = 0
                stats_buf[0] = None

            cur_slot = -1
            t2blk = None
            sums = None
            for (s, w, t, off) in sched:
                if pend_n[0] + t > DG:
                    flush()
                if s != cur_slot:
                    cur_slot = s
                    t2blk = iop.tile([P, 192], bf16, tag='t2blk')
                    nc.sync.dma_start(
                        out=t2blk[:], in_=t2x[s * P:(s + 1) * P, :])
                    sums = pps.tile([P, 192], f32, space='PSUM', tag='sums')
                wrows = min(WSZ, E_TBL - w * WSZ)
                gbuf = wk.tile([P, t * 192], f32, tag='gbuf')
                nc.gpsimd.dma_gather(
                    out_ap=gbuf[:].rearrange('p (t d) -> p t d', t=t),
                    in_ap=t1c[w * WSZ:w * WSZ + wrows, :],
                    idxs_ap=idx_all[:, off * 8:(off + t) * 8],
                    num_idxs=t * P,
                    num_idxs_reg=t * P,
                    elem_size=192,
                    single_packet=(t * P <= 512),
                )
                eftg = wk.tile([H, t * P], bf16, tag='eftg')
                nc.sync.dma_start(
                    out=eftg[:], in_=efT[:, off * P:(off + t) * P])
                efg = efp.tile([P, t * H], bf16, tag='efg')
                nc.sync.dma_start(
                    out=efg[:].rearrange('p (t f) -> p t f', f=H),
                    in_=ef_em[off * P:(off + t) * P, :].rearrange(
                        '(t p) f -> p t f', p=P))

                # batched one-hot for the group
                onehot = wk.tile([P, t * P], bf16, tag='onehot')
                nc.vector.tensor_tensor(
                    out=onehot[:].rearrange('p (t q) -> p t q', q=P),
                    in0=dl_all[:, off:off + t, None].to_broadcast([P, t, P]),
                    in1=iota_sb[:, None, :].to_broadcast([P, t, P]),
                    op=ALU.is_equal)
                trps = pp.tile([P, t * P], bf16, space='PSUM', tag='tr')
                for k in range(t):
                    nc.tensor.transpose(
                        out=trps[:, k * P:(k + 1) * P],
                        in_=onehot[:, k * P:(k + 1) * P],
                        identity=id_bf[:])
                ohne = wk.tile([P, t * P], bf16, tag='ohne')
                nc.vector.tensor_copy(out=ohne[:], in_=trps[:])

                mp = pp.tile([P, t * H], f32, space='PSUM', tag='mm')
                for k in range(t):
                    nc.tensor.matmul(
                        out=mp[:, k * H:(k + 1) * H],
                        lhsT=eftg[:, k * P:(k + 1) * P],
                        rhs=weg_sb[:], start=True, stop=False)
                    nc.tensor.matmul(
                        out=mp[:, k * H:(k + 1) * H],
                        lhsT=ohne[:, k * P:(k + 1) * P],
                        rhs=t2blk[:, 0:H], start=False, stop=True)
                msb = msp.tile([P, t * H], f32, tag='msb')
                nc.vector.tensor_add(
                    out=msb[:].rearrange('p (t f) -> p t f', f=H),
                    in0=mp[:].rearrange('p (t f) -> p t f', f=H),
                    in1=gbuf[:].rearrange('p (t d) -> p t d', d=192)[:, :, 0:H])

                valcat = wk.tile([P, t * 192], bf16, tag='valcat')
                vv = valcat[:].rearrange('p (t d) -> p t d', d=192)
                nc.scalar.activation(
                    out=vv[:, :, 0:H],
                    in_=msb[:].rearrange('p (t f) -> p t f', f=H),
                    func=AF.Sigmoid)
                nc.vector.tensor_tensor(
                    out=vv[:, :, H:192],
                    in0=gbuf[:].rearrange('p (t d) -> p t d', d=192)[:, :, H:192],
                    in1=vv[:, :, 0:H], op=ALU.mult)

                for k in range(t):
                    tt = off + k
                    nc.tensor.matmul(
                        out=sums[:],
                        lhsT=onehot[:, k * P:(k + 1) * P],
                        rhs=valcat[:, k * 192:(k + 1) * 192],
                        start=slot_first[s],
                        stop=(tt == slot_last[s]))
                    slot_first[s] = False

                if stats_buf[0] is None:
                    st_new = grp.tile([P, DG * 6], f32, tag='stats')
                    stats_buf[0] = st_new
                j = pend_n[0]
                for k0 in range(t):
                    nc.vector.bn_stats(
                        out=stats_buf[0][:, (j + k0) * 6:(j + k0 + 1) * 6],
                        in_=msb[:, k0 * H:(k0 + 1) * H])
                pending.append((off, t, msb, efg))
                pend_n[0] = j + t

                if off + t - 1 == slot_last[s]:
                    # xpre for slot s (LN deferred to final phase)
                    ssd = wk.tile([P, H], f32, tag='ssd')
                    nc.vector.tensor_scalar_add(
                        out=ssd[:], in0=sums[:, 0:H], scalar1=1e-6)
                    rec = wk.tile([P, H], f32, tag='rec')
                    nc.vector.reciprocal(out=rec[:], in_=ssd[:])
                    h = wk.tile([P, H], f32, tag='h')
                    nc.vector.tensor_mul(
                        out=h[:], in0=sums[:, H:192], in1=rec[:])
                    nc.vector.tensor_add(
                        out=xpre_all[:, s * H:(s + 1) * H],
                        in0=h[:], in1=t2blk[:, H:192])
            flush()

            # ---- final phase: node LN + silu + residual ----
            for s0 in range(0, S, DG):
                g = min(DG, S - s0)
                st = grp.tile([P, DG * 6], f32, tag='stats')
                for k0 in range(g):
                    nc.vector.bn_stats(
                        out=st[:, k0 * 6:(k0 + 1) * 6],
                        in_=xpre_all[:, (s0 + k0) * H:(s0 + k0 + 1) * H])
                rstd, nmr = ln_coeffs(st, g)
                nfblk = ybp.tile([P, DG * H], f32, tag='nfblk')
                nc.sync.dma_start(
                    out=nfblk[:, :g * H].rearrange('p (g f) -> p g f', f=H),
                    in_=nfb[s0 * P:(s0 + g) * P, :].rearrange(
                        '(g p) f -> p g f', p=P))
                xbuf = ybp.tile([P, DG * H], f32, tag='ybuf')
                for k in range(g):
                    s = s0 + k
                    nc.scalar.activation(
                        out=xbuf[:, k * H:(k + 1) * H],
                        in_=xpre_all[:, s * H:(s + 1) * H],
                        func=AF.Silu, bias=nmr[:, k:k + 1],
                        scale=rstd[:, k:k + 1])
                nc.vector.tensor_add(
                    out=xbuf[:, :g * H], in0=xbuf[:, :g * H],
                    in1=nfblk[:, :g * H])
                nc.sync.dma_start(
                    out=xout[s0 * P:(s0 + g) * P, :].rearrange(
                        '(g p) f -> p g f', p=P),
                    in_=xbuf[:, :g * H].rearrange('p (g f) -> p g f', f=H))

    nc.finalize()
    return nc


# ----------------------------------------------------------------------------
# top-level
# ----------------------------------------------------------------------------

_TRACE = [False]


def kernel(**inputs):
    from concourse.bass_utils import run_bass_kernel_spmd

    src = np.asarray(inputs['src'])
    dst = np.asarray(inputs['dst'])
    node_feats = np.asarray(inputs['node_feats'], np.float32)
    edge_feats = np.asarray(inputs['edge_feats'], np.float32)
    N, E = node_feats.shape[0], edge_feats.shape[0]

    plan = build_plan(src, dst, N)
    in_maps = build_inputs(plan, inputs)
    nc = build_kernel(plan)
    res = run_bass_kernel_spmd(
        nc, in_maps, core_ids=list(range(NCORES)), trace=_TRACE[0])
    kernel.last_result = res

    x = np.zeros((N, H), np.float32)
    y = np.zeros((E, H), np.float32)
    for c in range(NCORES):
        out = res.results[c]
        blocks = plan['slot_block'][c]
        xs = out['xout'].reshape(plan['S'], P, H)
        for s_i, b in enumerate(blocks):
            lo = b * P
            hi = min(lo + P, N)
            if lo < N:
                x[lo:hi] = xs[s_i, :hi - lo]
        canon = plan['canon_edge'][c]
        real = canon >= 0
        y[canon[real]] = out['y_em'][real]
    return x, y


# revision 22
# speedup vs baseline: 3.9628x; 1.0021x over previous
"""ALIGNN edge-gated message passing on 8 Trainium2 NeuronCores.

Strategy: edges partitioned by dst-block across cores (no collectives).
Each core receives host-prepared, per-core data:
  - a compacted node table (only nodes referenced as src by its edges)
  - its own 98 node-blocks (128 nodes each) in load-balanced "slot" order
  - its edge slice in canonical (slot, window, tile) order
Device work per core:
  phase A : PE matmuls node_feats -> T1=[e_src+bias | Bh] (f32, compact rows)
            and T2x=[e_dst | x_lin] (bf16, own blocks)
  phase B : per (slot,window) group of 128-edge tiles: dma_gather T1 rows by
            src (int16 windows), one-hot(dst_local) via batched is_equal,
            m = ef@W_eg + onehot.T@T2blk (+ gathered), sigma = sigmoid(m),
            segment-sum via one-hot matmul accumulation in PSUM;
            LN sqrt batched over deferred groups of 32 tiles, then
            y = ef + silu(LN(m)) written edge-major
  phase D : per slot: xpre = x_lin + ssh/(ss+1e-6); final phase applies
            LN + silu + residual for all slots with batched stats
"""
import sys

if '/opt/trn_rl_repo' not in sys.path:
    sys.path.insert(0, '/opt/trn_rl_repo')

import numpy as np
import ml_dtypes

BF16 = ml_dtypes.bfloat16
H = 96
LN_EPS = 1e-5
NCORES = 8
P = 128
WMAX = 32640          # gather window rows (<= int16 max, mult of 128)
DG = 32               # deferred-LN group size (tiles)


# ----------------------------------------------------------------------------
# host-side plan
# ----------------------------------------------------------------------------

def build_plan(src, dst, N):
    E = src.shape[0]
    n_blocks_real = (N + P - 1) // P
    n_blocks = ((n_blocks_real + NCORES - 1) // NCORES) * NCORES
    S = n_blocks // NCORES              # slots per core
    N_pad = n_blocks * P

    blk_of_edge = dst // P
    blk_counts = np.bincount(blk_of_edge, minlength=n_blocks)

    # balanced assignment: sort blocks by count desc, greedily fill cores
    order = np.argsort(-blk_counts, kind='stable')
    core_load = np.zeros(NCORES, dtype=np.int64)
    core_nblk = np.zeros(NCORES, dtype=np.int64)
    blk_core = np.zeros(n_blocks, dtype=np.int64)
    for b in order:
        cands = np.where(core_nblk < S)[0]
        c = cands[np.argmin(core_load[cands])]
        blk_core[b] = c
        core_load[c] += blk_counts[b]
        core_nblk[c] += 1

    # per-core slot order: blocks sorted by count desc
    slot_block = np.zeros((NCORES, S), dtype=np.int64)   # slot -> block id
    for c in range(NCORES):
        blks = np.where(blk_core == c)[0]
        blks = blks[np.argsort(-blk_counts[blks], kind='stable')]
        slot_block[c] = blks

    edge_core = blk_core[blk_of_edge]
    slot_of_block = np.zeros(n_blocks, dtype=np.int64)
    for c in range(NCORES):
        slot_of_block[slot_block[c]] = np.arange(S)
    edge_slot = slot_of_block[blk_of_edge]

    # compacted src table per core
    srclist = []
    for c in range(NCORES):
        u = np.unique(src[edge_core == c])
        srclist.append(u)
    E_TBL = ((max(len(u) for u in srclist) + P - 1) // P) * P
    n_win = max(1, (E_TBL + WMAX - 1) // WMAX)
    WSZ = ((E_TBL // n_win + P - 1) // P) * P
    assert WSZ <= 32767

    src_pos = np.zeros((NCORES, E), dtype=np.int64)
    for c in range(NCORES):
        m = edge_core == c
        src_pos[c, m] = np.searchsorted(srclist[c], src[m])

    cnt = np.zeros((NCORES, S, n_win), dtype=np.int64)
    for c in range(NCORES):
        m = edge_core == c
        w = src_pos[c, m] // WSZ
        np.add.at(cnt[c], (edge_slot[m], w), 1)
    tiles_sw = np.maximum(np.ceil(cnt / P).astype(np.int64).max(axis=0), 0)
    tiles_sw[:, 0] = np.maximum(tiles_sw[:, 0], 1)   # every slot >=1 tile
    TT = int(tiles_sw.sum())
    E_pad = TT * P

    sched = []
    off = 0
    TMAXG = 5
    for s in range(S):
        for w in range(n_win):
            t = int(tiles_sw[s, w])
            while t > 0:
                tc_ = min(t, TMAXG)
                sched.append((s, w, tc_, off))
                off += tc_
                t -= tc_

    canon_edge = np.full((NCORES, E_pad), -1, dtype=np.int64)
    for c in range(NCORES):
        m = np.where(edge_core == c)[0]
        w = src_pos[c, m] // WSZ
        key = edge_slot[m] * n_win + w
        ordr = np.argsort(key, kind='stable')
        me, ke = m[ordr], key[ordr]
        group_off = {}
        for (s_, w_, t_, o_) in sched:
            if (s_, w_) not in group_off:
                group_off[(s_, w_)] = o_ * P
        pos = np.zeros(len(me), dtype=np.int64)
        start = 0
        for k in np.unique(ke):
            cnt_k = int((ke == k).sum())
            s_, w_ = divmod(int(k), n_win)
            base = group_off[(s_, w_)]
            pos[start:start + cnt_k] = base + np.arange(cnt_k)
            start += cnt_k
        canon_edge[c, pos] = me
    return dict(
        N_pad=N_pad, n_blocks=n_blocks, S=S, E_TBL=E_TBL, n_win=n_win,
        WSZ=WSZ, TT=TT, E_pad=E_pad, sched=sched, slot_block=slot_block,
        srclist=srclist, src_pos=src_pos, canon_edge=canon_edge,
    )


def build_inputs(plan, inputs):
    node_feats = np.asarray(inputs['node_feats'], np.float32)
    edge_feats = np.asarray(inputs['edge_feats'], np.float32)
    src = np.asarray(inputs['src'])
    dst = np.asarray(inputs['dst'])
    N = node_feats.shape[0]

    tp = (np.asarray(inputs['time_feats'], np.float32) @
          np.asarray(inputs['W_tp'], np.float32) +
          np.asarray(inputs['b_tp'], np.float32))[0]
    bias_src = np.asarray(inputs['b_sg'], np.float32) + tp + \
        np.asarray(inputs['b_eg'], np.float32)

    W1b = np.concatenate([
        np.concatenate([inputs['W_sg'], inputs['W_du']], axis=1),
        np.concatenate([bias_src, inputs['b_du']])[None, :],
    ], axis=0).astype(np.float32)                      # [97, 192]
    W2b = np.concatenate([
        np.concatenate([inputs['W_dg'], inputs['W_su']], axis=1),
        np.concatenate([inputs['b_dg'], inputs['b_su']])[None, :],
    ], axis=0).astype(np.float32)                      # [97, 192]

    S, E_TBL, E_pad, TT = plan['S'], plan['E_TBL'], plan['E_pad'], plan['TT']
    nf_pad = np.zeros((plan['N_pad'], H), np.float32)
    nf_pad[:N] = node_feats

    iota = np.tile(np.arange(P, dtype=np.float32), (P, 1))
    ident = np.eye(P, dtype=np.float32)

    in_maps = []
    for c in range(NCORES):
        u = plan['srclist'][c]
        nftc = np.zeros((97, E_TBL), np.float32)
        nftc[:H, :len(u)] = node_feats[u].T
        nftc[96, :] = 1.0

        blocks = plan['slot_block'][c]
        own = nf_pad.reshape(-1, P, H)[blocks]          # [S, 128, 96]
        own_flat = own.reshape(S * P, H)
        nfbT = np.zeros((97, S * P), np.float32)
        nfbT[:H] = own_flat.T
        nfbT[96] = 1.0

        canon = plan['canon_edge'][c]
        real = canon >= 0
        ef_can = np.zeros((E_pad, H), np.float32)
        ef_can[real] = edge_feats[canon[real]]

        dstloc = np.full(E_pad, -1.0, np.float32)
        dstloc[real] = (dst[canon[real]] % P).astype(np.float32)
        dstloc = dstloc.reshape(TT, P).T.copy()         # [128, TT]

        gpos = np.zeros(E_pad, np.int64)
        gpos[real] = plan['src_pos'][c, canon[real]] % plan['WSZ']
        gidx = np.zeros((16, E_pad // 16), np.int16)
        idx_lin = np.arange(E_pad)
        gidx[idx_lin % 16, idx_lin // 16] = gpos.astype(np.int16)
        gidx = np.tile(gidx, (8, 1))                    # [128, E_pad/16]

        in_maps.append({
            'nftc': nftc.astype(BF16),
            'nfbT': nfbT.astype(BF16),
            'w1b': W1b.astype(BF16), 'w2b': W2b.astype(BF16),
            'weg': np.asarray(inputs['W_eg'], np.float32).astype(BF16),
            'efT': ef_can.T.astype(BF16).copy(),
            'ef_em': ef_can.astype(BF16),
            'dstloc': dstloc,
            'gidx': gidx,
            'iota': iota,
            'ident': ident.astype(BF16),
            'nfb': own_flat,
        })
    return in_maps


# ----------------------------------------------------------------------------
# device kernel
# ----------------------------------------------------------------------------

def build_kernel(plan):
    import concourse.bacc as bacc
    import concourse.bass as bass
    import concourse.mybir as mybir
    import concourse.tile as tile

    f32, bf16, i16 = mybir.dt.float32, mybir.dt.bfloat16, mybir.dt.int16
    AF = mybir.ActivationFunctionType
    ALU = mybir.AluOpType

    S, E_TBL, E_pad, TT = plan['S'], plan['E_TBL'], plan['E_pad'], plan['TT']
    n_win, WSZ = plan['n_win'], plan['WSZ']
    sched = plan['sched']
    NB = S * P

    nc = bacc.Bacc()
    dp = nc.declare_dram_parameter
    nftc = dp('nftc', [97, E_TBL], bf16, isOutput=False)
    nfbT = dp('nfbT', [97, NB], bf16, isOutput=False)
    w1b = dp('w1b', [97, 192], bf16, isOutput=False)
    w2b = dp('w2b', [97, 192], bf16, isOutput=False)
    weg = dp('weg', [H, H], bf16, isOutput=False)
    efT = dp('efT', [H, E_pad], bf16, isOutput=False)
    ef_em = dp('ef_em', [E_pad, H], bf16, isOutput=False)
    dstloc = dp('dstloc', [P, TT], f32, isOutput=False)
    gidx = dp('gidx', [P, E_pad // 16], i16, isOutput=False)
    iota = dp('iota', [P, P], f32, isOutput=False)
    ident = dp('ident', [P, P], bf16, isOutput=False)
    nfb = dp('nfb', [NB, H], f32, isOutput=False)
    y_em = dp('y_em', [E_pad, H], bf16, isOutput=True)
    xout = dp('xout', [NB, H], f32, isOutput=True)

    t1c = nc.dram_tensor('t1c', [E_TBL, 256], bf16)
    t2x = nc.dram_tensor('t2x', [NB, 192], bf16)

    with tile.TileContext(nc) as tc:
        with (
            tc.tile_pool(name='const', bufs=1) as cpool,
            tc.tile_pool(name='io', bufs=4) as iop,
            tc.tile_pool(name='pa', bufs=2) as pa,
            tc.tile_pool(name='eft', bufs=2) as efp,
            tc.tile_pool(name='msb', bufs=10) as msp,
            tc.tile_pool(name='work', bufs=4) as wk,
            tc.tile_pool(name='grp', bufs=2) as grp,
            tc.tile_pool(name='yb', bufs=2) as ybp,
            tc.tile_pool(name='ps', bufs=3, space='PSUM') as pp,
            tc.tile_pool(name='ps_sum', bufs=2, space='PSUM') as pps,
        ):
            # ---- constants ----
            iota_sb = cpool.tile([P, P], f32, tag='iota')
            nc.sync.dma_start(out=iota_sb[:], in_=iota[:])
            id_bf = cpool.tile([P, P], bf16, tag='idb')
            nc.sync.dma_start(out=id_bf[:], in_=ident[:])
            w1_sb = cpool.tile([97, 192], bf16, tag='w1')
            nc.sync.dma_start(out=w1_sb[:], in_=w1b[:])
            w2_sb = cpool.tile([97, 192], bf16, tag='w2')
            nc.sync.dma_start(out=w2_sb[:], in_=w2b[:])
            weg_sb = cpool.tile([H, H], bf16, tag='weg')
            nc.sync.dma_start(out=weg_sb[:], in_=weg[:])
            idx_all = cpool.tile([P, E_pad // 16], i16, tag='gidx')
            nc.sync.dma_start(out=idx_all[:], in_=gidx[:])
            dl_all = cpool.tile([P, TT], f32, tag='dstloc')
            nc.sync.dma_start(out=dl_all[:], in_=dstloc[:])
            eps_col = cpool.tile([P, 1], f32, tag='eps')
            nc.vector.memset(eps_col[:], LN_EPS)
            xpre_all = cpool.tile([P, S * H], bf16, tag='xpre')

            # ---- phase A: node transform tables ----
            ACH = 16
            for (tbl, srcT, wsb, n_tiles, dcols) in (
                (t1c, nftc, w1_sb, E_TBL // P, 256),
                (t2x, nfbT, w2_sb, S, 192),
            ):
                for j0 in range(0, n_tiles, ACH):
                    jn = min(ACH, n_tiles - j0)
                    nchunk = pa.tile([97, ACH * P], bf16, tag='nfa')
                    nc.sync.dma_start(
                        out=nchunk[:, :jn * P],
                        in_=srcT[:, j0 * P:(j0 + jn) * P])
                    tbuf = pa.tile([P, ACH * 256], bf16, tag='tbuf')
                    if dcols == 256:
                        nc.vector.memset(
                            tbuf[:].rearrange(
                                'p (j d) -> p j d', d=256)[:, :, 192:256], 0)
                    for k in range(jn):
                        mm = pp.tile([P, 192], f32, space='PSUM', tag='mm')
                        nc.tensor.matmul(
                            out=mm[:], lhsT=nchunk[:, k * P:(k + 1) * P],
                            rhs=wsb[:], start=True, stop=True)
                        nc.vector.tensor_copy(
                            out=tbuf[:, k * dcols:k * dcols + 192], in_=mm[:])
                    nc.sync.dma_start(
                        out=tbl[j0 * P:(j0 + jn) * P, :].rearrange(
                            '(j p) d -> p j d', p=P),
                        in_=tbuf[:, :jn * dcols].rearrange(
                            'p (j d) -> p j d', d=dcols))

            # ---- phase B ----
            slot_first = {s: True for s in range(S)}
            slot_last = {}
            for (s, w, t, off) in sched:
                slot_last[s] = off + t - 1

            pending = []     # (off, t, msb, efg) per (s,w) group
            pend_n = [0]
            stats_buf = [None]

            def ln_coeffs(st, g):
                """Batched LN: stats [P, g, 6] -> (rstd, nmr) [P, g]."""
                stv = st[:].rearrange('p (g s) -> p g s', s=6)
                a1 = grp.tile([P, DG], f32, tag='a1')
                nc.vector.tensor_add(
                    out=a1[:, :g], in0=stv[:, :g, 2], in1=stv[:, :g, 5])
                a2 = grp.tile([P, DG], f32, tag='a2')
                nc.vector.tensor_sub(
                    out=a2[:, :g], in0=stv[:, :g, 1], in1=stv[:, :g, 4])
                a3 = grp.tile([P, DG], f32, tag='a3')
                nc.vector.tensor_mul(
                    out=a3[:, :g], in0=a2[:, :g], in1=a2[:, :g])
                var = grp.tile([P, DG], f32, tag='var')
                nc.vector.tensor_scalar(
                    out=var[:, :g], in0=a1[:, :g], scalar1=1.0 / 96.0,
                    scalar2=None, op0=ALU.mult)
                nc.vector.tensor_scalar(
                    out=a3[:, :g], in0=a3[:, :g], scalar1=0.25,
                    scalar2=None, op0=ALU.mult)
                nc.vector.tensor_add(
                    out=var[:, :g], in0=var[:, :g], in1=a3[:, :g])
                std = grp.tile([P, DG], f32, tag='std')
                nc.scalar.activation(
                    out=std[:, :g], in_=var[:, :g], func=AF.Sqrt,
                    bias=eps_col[:])
                rstd = grp.tile([P, DG], f32, tag='rstd')
                nc.vector.reciprocal(out=rstd[:, :g], in_=std[:, :g])
                msum = grp.tile([P, DG], f32, tag='msum')
                nc.vector.tensor_add(
                    out=msum[:, :g], in0=stv[:, :g, 1], in1=stv[:, :g, 4])
                nmr = grp.tile([P, DG], f32, tag='nmr')
                nc.vector.tensor_mul(
                    out=nmr[:, :g], in0=msum[:, :g], in1=rstd[:, :g])
                nc.vector.tensor_scalar(
                    out=nmr[:, :g], in0=nmr[:, :g], scalar1=-0.5,
                    scalar2=None, op0=ALU.mult)
                return rstd, nmr

            def flush():
                if not pending:
                    return
                g = pend_n[0]
                rstd, nmr = ln_coeffs(stats_buf[0], g)
                ybuf = ybp.tile([P, DG * H], bf16, tag='ybuf')
                j = 0
                off0 = pending[0][0]
                for (off_, t_, msb_, efg_) in pending:
                    for k in range(t_):
                        nc.scalar.activation(
                            out=ybuf[:, (j + k) * H:(j + k + 1) * H],
                            in_=msb_[:, k * H:(k + 1) * H],
                            func=AF.Silu,
                            bias=nmr[:, j + k:j + k + 1],
                            scale=rstd[:, j + k:j + k + 1])
                    nc.vector.tensor_add(
                        out=ybuf[:, j * H:(j + t_) * H],
                        in0=ybuf[:, j * H:(j + t_) * H],
                        in1=efg_)
                    j += t_
                nc.sync.dma_start(
                    out=y_em[off0 * P:(off0 + g) * P, :].rearrange(
                        '(j p) f -> p j f', p=P),
                    in_=ybuf[:, :g * H].rearrange('p (j f) -> p j f', f=H))
                pending.clear()
                pend_n[0] = 0
                stats_buf[0] = None

            cur_slot = -1
            cur_span = None
            t2base = 0
            sums = None
            win_base = [None]
            win_len = [0]
            win_eftg = [None]
            win_efg = [None]
            for (s, w, t, off) in sched:
                if pend_n[0] + t > DG:
                    flush()
                if s != cur_slot:
                    if s % 8 == 0:
                        t2span = iop.tile([P, 8 * 192], bf16, tag='t2span')
                        sn = min(8, S - s)
                        nc.sync.dma_start(
                            out=t2span[:, :sn * 192].rearrange(
                                'p (j d) -> p j d', d=192),
                            in_=t2x[s * P:(s + sn) * P, :].rearrange(
                                '(j p) d -> p j d', p=P))
                        cur_span = t2span
                    cur_slot = s
                    t2base = (s % 8) * 192
                    sums = pps.tile([P, 192], f32, space='PSUM', tag='sums')
                wrows = min(WSZ, E_TBL - w * WSZ)
                gbuf = wk.tile([P, t * 256], bf16, tag='gbuf')
                nc.gpsimd.dma_gather(
                    out_ap=gbuf[:].rearrange('p (t d) -> p t d', t=t),
                    in_ap=t1c[w * WSZ:w * WSZ + wrows, :],
                    idxs_ap=idx_all[:, off * 8:(off + t) * 8],
                    num_idxs=t * P,
                    num_idxs_reg=t * P,
                    elem_size=256,
                    single_packet=(t * P <= 512),
                )
                if win_base[0] is None or off >= win_base[0] + win_len[0]:
                    wb = off
                    wl = 0
                    for (s2, w2, t2, off2) in sched:
                        if off2 < wb:
                            continue
                        if wl + t2 > DG:
                            break
                        wl += t2
                    win_base[0] = wb
                    win_len[0] = wl
                    eftg_w = efp.tile([H, DG * P], bf16, tag='eftgw')
                    nc.sync.dma_start(
                        out=eftg_w[:, :wl * P],
                        in_=efT[:, wb * P:(wb + wl) * P])
                    efg_w = efp.tile([P, DG * H], bf16, tag='efgw')
                    nc.sync.dma_start(
                        out=efg_w[:, :wl * H].rearrange(
                            'p (t f) -> p t f', f=H),
                        in_=ef_em[wb * P:(wb + wl) * P, :].rearrange(
                            '(t p) f -> p t f', p=P))
                    win_eftg[0] = eftg_w
                    win_efg[0] = efg_w
                lo = off - win_base[0]

                # batched one-hot for the group
                onehot = wk.tile([P, t * P], bf16, tag='onehot')
                nc.vector.tensor_tensor(
                    out=onehot[:].rearrange('p (t q) -> p t q', q=P),
                    in0=dl_all[:, off:off + t, None].to_broadcast([P, t, P]),
                    in1=iota_sb[:, None, :].to_broadcast([P, t, P]),
                    op=ALU.is_equal)
                trps = pp.tile([P, t * P], bf16, space='PSUM', tag='tr')
                for k in range(t):
                    nc.tensor.transpose(
                        out=trps[:, k * P:(k + 1) * P],
                        in_=onehot[:, k * P:(k + 1) * P],
                        identity=id_bf[:])
                ohne = wk.tile([P, t * P], bf16, tag='ohne')
                nc.vector.tensor_copy(out=ohne[:], in_=trps[:])

                mp = pp.tile([P, t * H], f32, space='PSUM', tag='mm')
                for k in range(t):
                    nc.tensor.matmul(
                        out=mp[:, k * H:(k + 1) * H],
                        lhsT=win_eftg[0][:, (lo + k) * P:(lo + k + 1) * P],
                        rhs=weg_sb[:], start=True, stop=False)
                    nc.tensor.matmul(
                        out=mp[:, k * H:(k + 1) * H],
                        lhsT=ohne[:, k * P:(k + 1) * P],
                        rhs=cur_span[:, t2base:t2base + H],
                        start=False, stop=True)
                msb = msp.tile([P, t * H], f32, tag='msb')
                nc.vector.tensor_add(
                    out=msb[:].rearrange('p (t f) -> p t f', f=H),
                    in0=mp[:].rearrange('p (t f) -> p t f', f=H),
                    in1=gbuf[:].rearrange('p (t d) -> p t d', d=256)[:, :, 0:H])

                valcat = wk.tile([P, t * 192], bf16, tag='valcat')
                vv = valcat[:].rearrange('p (t d) -> p t d', d=192)
                nc.scalar.activation(
                    out=vv[:, :, 0:H],
                    in_=msb[:].rearrange('p (t f) -> p t f', f=H),
                    func=AF.Sigmoid)
                nc.vector.tensor_tensor(
                    out=vv[:, :, H:192],
                    in0=gbuf[:].rearrange('p (t d) -> p t d', d=256)[:, :, H:192],
                    in1=vv[:, :, 0:H], op=ALU.mult)

                for k in range(t):
                    tt = off + k
                    nc.tensor.matmul(
                        out=sums[:],
                        lhsT=onehot[:, k * P:(k + 1) * P],
                        rhs=valcat[:, k * 192:(k + 1) * 192],
                        start=slot_first[s],
                        stop=(tt == slot_last[s]))
                    slot_first[s] = False

                if stats_buf[0] is None:
                    st_new = grp.tile([P, DG * 6], f32, tag='stats')
                    stats_buf[0] = st_new
                j = pend_n[0]
                for k0 in range(t):
                    nc.vector.bn_stats(
                        out=stats_buf[0][:, (j + k0) * 6:(j + k0 + 1) * 6],
                        in_=msb[:, k0 * H:(k0 + 1) * H])
                pending.append((off, t, msb,
                                win_efg[0][:, lo * H:(lo + t) * H]))
                pend_n[0] = j + t

                if off + t - 1 == slot_last[s]:
                    # xpre for slot s (LN deferred to final phase)
                    ssd = wk.tile([P, H], f32, tag='ssd')
                    nc.vector.tensor_scalar_add(
                        out=ssd[:], in0=sums[:, 0:H], scalar1=1e-6)
                    rec = wk.tile([P, H], f32, tag='rec')
                    nc.vector.reciprocal(out=rec[:], in_=ssd[:])
                    h = wk.tile([P, H], f32, tag='h')
                    nc.vector.tensor_mul(
                        out=h[:], in0=sums[:, H:192], in1=rec[:])
                    nc.vector.tensor_add(
                        out=xpre_all[:, s * H:(s + 1) * H],
                        in0=h[:], in1=cur_span[:, t2base + H:t2base + 192])
            flush()

            # ---- final phase: node LN + silu + residual ----
            for s0 in range(0, S, DG):
                g = min(DG, S - s0)
                st = grp.tile([P, DG * 6], f32, tag='stats')
                for k0 in range(g):
                    nc.vector.bn_stats(
                        out=st[:, k0 * 6:(k0 + 1) * 6],
                        in_=xpre_all[:, (s0 + k0) * H:(s0 + k0 + 1) * H])
                rstd, nmr = ln_coeffs(st, g)
                nfblk = ybp.tile([P, DG * H], f32, tag='nfblk')
                nc.sync.dma_start(
                    out=nfblk[:, :g * H].rearrange('p (g f) -> p g f', f=H),
                    in_=nfb[s0 * P:(s0 + g) * P, :].rearrange(
                        '(g p) f -> p g f', p=P))
                xbuf = ybp.tile([P, DG * H], f32, tag='ybuf')
                for k in range(g):
                    s = s0 + k
                    nc.scalar.activation(
                        out=xbuf[:, k * H:(k + 1) * H],
                        in_=xpre_all[:, s * H:(s + 1) * H],
                        func=AF.Silu, bias=nmr[:, k:k + 1],
                        scale=rstd[:, k:k + 1])
                nc.vector.tensor_add(
                    out=xbuf[:, :g * H], in0=xbuf[:, :g * H],
                    in1=nfblk[:, :g * H])
                nc.sync.dma_start(
                    out=xout[s0 * P:(s0 + g) * P, :].rearrange(
                        '(g p) f -> p g f', p=P),
                    in_=xbuf[:, :g * H].rearrange('p (g f) -> p g f', f=H))

    nc.finalize()
    return nc


# ----------------------------------------------------------------------------
# top-level
# ----------------------------------------------------------------------------

_TRACE = [False]


def kernel(**inputs):
    from concourse.bass_utils import run_bass_kernel_spmd

    src = np.asarray(inputs['src'])
    dst = np.asarray(inputs['dst'])
    node_feats = np.asarray(inputs['node_feats'], np.float32)
    edge_feats = np.asarray(inputs['edge_feats'], np.float32)
    N, E = node_feats.shape[0], edge_feats.shape[0]

    plan = build_plan(src, dst, N)
    in_maps = build_inputs(plan, inputs)
    nc = build_kernel(plan)
    res = run_bass_kernel_spmd(
        nc, in_maps, core_ids=list(range(NCORES)), trace=_TRACE[0])
    kernel.last_result = res

    x = np.zeros((N, H), np.float32)
    y = np.zeros((E, H), np.float32)
    for c in range(NCORES):
        out = res.results[c]
        blocks = plan['slot_block'][c]
        xs = out['xout'].reshape(plan['S'], P, H)
        for s_i, b in enumerate(blocks):
            lo = b * P
            hi = min(lo + P, N)
            if lo < N:
                x[lo:hi] = xs[s_i, :hi - lo]
        canon = plan['canon_edge'][c]
        real = canon >= 0
        y[canon[real]] = out['y_em'][real]
    return x, y


# revision 23
# speedup vs baseline: 4.0239x; 1.0154x over previous
"""ALIGNN edge-gated message passing on 8 Trainium2 NeuronCores.

Strategy: edges partitioned by dst-block across cores (no collectives).
Each core receives host-prepared, per-core data:
  - a compacted node table (only nodes referenced as src by its edges)
  - its own 98 node-blocks (128 nodes each) in load-balanced "slot" order
  - its edge slice in canonical (slot, window, tile) order
Device work per core:
  phase A : PE matmuls node_feats -> T1=[e_src+bias | Bh] (f32, compact rows)
            and T2x=[e_dst | x_lin] (bf16, own blocks)
  phase B : per (slot,window) group of 128-edge tiles: dma_gather T1 rows by
            src (int16 windows), one-hot(dst_local) via batched is_equal,
            m = ef@W_eg + onehot.T@T2blk (+ gathered), sigma = sigmoid(m),
            segment-sum via one-hot matmul accumulation in PSUM;
            LN sqrt batched over deferred groups of 32 tiles, then
            y = ef + silu(LN(m)) written edge-major
  phase D : per slot: xpre = x_lin + ssh/(ss+1e-6); final phase applies
            LN + silu + residual for all slots with batched stats
"""
import sys

if '/opt/trn_rl_repo' not in sys.path:
    sys.path.insert(0, '/opt/trn_rl_repo')

import numpy as np
import ml_dtypes

BF16 = ml_dtypes.bfloat16
H = 96
LN_EPS = 1e-5
NCORES = 8
P = 128
WMAX = 32640          # gather window rows (<= int16 max, mult of 128)
DG = 32               # deferred-LN group size (tiles)


# ----------------------------------------------------------------------------
# host-side plan
# ----------------------------------------------------------------------------

def build_plan(src, dst, N):
    E = src.shape[0]
    n_blocks_real = (N + P - 1) // P
    n_blocks = ((n_blocks_real + NCORES - 1) // NCORES) * NCORES
    S = n_blocks // NCORES              # slots per core
    N_pad = n_blocks * P

    blk_of_edge = dst // P
    blk_counts = np.bincount(blk_of_edge, minlength=n_blocks)

    # balanced assignment: sort blocks by count desc, greedily fill cores
    order = np.argsort(-blk_counts, kind='stable')
    core_load = np.zeros(NCORES, dtype=np.int64)
    core_nblk = np.zeros(NCORES, dtype=np.int64)
    blk_core = np.zeros(n_blocks, dtype=np.int64)
    for b in order:
        cands = np.where(core_nblk < S)[0]
        c = cands[np.argmin(core_load[cands])]
        blk_core[b] = c
        core_load[c] += blk_counts[b]
        core_nblk[c] += 1

    # per-core slot order: blocks sorted by count desc
    slot_block = np.zeros((NCORES, S), dtype=np.int64)   # slot -> block id
    for c in range(NCORES):
        blks = np.where(blk_core == c)[0]
        blks = blks[np.argsort(-blk_counts[blks], kind='stable')]
        slot_block[c] = blks

    edge_core = blk_core[blk_of_edge]
    slot_of_block = np.zeros(n_blocks, dtype=np.int64)
    for c in range(NCORES):
        slot_of_block[slot_block[c]] = np.arange(S)
    edge_slot = slot_of_block[blk_of_edge]

    # compacted src table per core
    srclist = []
    for c in range(NCORES):
        u = np.unique(src[edge_core == c])
        srclist.append(u)
    E_TBL = ((max(len(u) for u in srclist) + P - 1) // P) * P
    n_win = max(1, (E_TBL + WMAX - 1) // WMAX)
    WSZ = ((E_TBL // n_win + P - 1) // P) * P
    assert WSZ <= 32767

    src_pos = np.zeros((NCORES, E), dtype=np.int64)
    for c in range(NCORES):
        m = edge_core == c
        src_pos[c, m] = np.searchsorted(srclist[c], src[m])

    cnt = np.zeros((NCORES, S, n_win), dtype=np.int64)
    for c in range(NCORES):
        m = edge_core == c
        w = src_pos[c, m] // WSZ
        np.add.at(cnt[c], (edge_slot[m], w), 1)
    tiles_sw = np.maximum(np.ceil(cnt / P).astype(np.int64).max(axis=0), 0)
    tiles_sw[:, 0] = np.maximum(tiles_sw[:, 0], 1)   # every slot >=1 tile
    TT = int(tiles_sw.sum())
    E_pad = TT * P

    sched = []
    off = 0
    TMAXG = 5
    for s in range(S):
        for w in range(n_win):
            t = int(tiles_sw[s, w])
            while t > 0:
                tc_ = min(t, TMAXG)
                sched.append((s, w, tc_, off))
                off += tc_
                t -= tc_

    canon_edge = np.full((NCORES, E_pad), -1, dtype=np.int64)
    for c in range(NCORES):
        m = np.where(edge_core == c)[0]
        w = src_pos[c, m] // WSZ
        key = edge_slot[m] * n_win + w
        ordr = np.argsort(key, kind='stable')
        me, ke = m[ordr], key[ordr]
        group_off = {}
        for (s_, w_, t_, o_) in sched:
            if (s_, w_) not in group_off:
                group_off[(s_, w_)] = o_ * P
        pos = np.zeros(len(me), dtype=np.int64)
        start = 0
        for k in np.unique(ke):
            cnt_k = int((ke == k).sum())
            s_, w_ = divmod(int(k), n_win)
            base = group_off[(s_, w_)]
            pos[start:start + cnt_k] = base + np.arange(cnt_k)
            start += cnt_k
        canon_edge[c, pos] = me
    return dict(
        N_pad=N_pad, n_blocks=n_blocks, S=S, E_TBL=E_TBL, n_win=n_win,
        WSZ=WSZ, TT=TT, E_pad=E_pad, sched=sched, slot_block=slot_block,
        srclist=srclist, src_pos=src_pos, canon_edge=canon_edge,
    )


def build_inputs(plan, inputs):
    node_feats = np.asarray(inputs['node_feats'], np.float32)
    edge_feats = np.asarray(inputs['edge_feats'], np.float32)
    src = np.asarray(inputs['src'])
    dst = np.asarray(inputs['dst'])
    N = node_feats.shape[0]

    tp = (np.asarray(inputs['time_feats'], np.float32) @
          np.asarray(inputs['W_tp'], np.float32) +
          np.asarray(inputs['b_tp'], np.float32))[0]
    bias_src = np.asarray(inputs['b_sg'], np.float32) + tp + \
        np.asarray(inputs['b_eg'], np.float32)

    W1b = np.concatenate([
        np.concatenate([inputs['W_sg'], inputs['W_du']], axis=1),
        np.concatenate([bias_src, inputs['b_du']])[None, :],
    ], axis=0).astype(np.float32)                      # [97, 192]
    W2b = np.concatenate([
        np.concatenate([inputs['W_dg'], inputs['W_su']], axis=1),
        np.concatenate([inputs['b_dg'], inputs['b_su']])[None, :],
    ], axis=0).astype(np.float32)                      # [97, 192]

    S, E_TBL, E_pad, TT = plan['S'], plan['E_TBL'], plan['E_pad'], plan['TT']
    nf_pad = np.zeros((plan['N_pad'], H), np.float32)
    nf_pad[:N] = node_feats

    iota = np.tile(np.arange(P, dtype=np.float32), (P, 1))
    ident = np.eye(P, dtype=np.float32)

    in_maps = []
    for c in range(NCORES):
        u = plan['srclist'][c]
        nftc = np.zeros((97, E_TBL), np.float32)
        nftc[:H, :len(u)] = node_feats[u].T
        nftc[96, :] = 1.0

        blocks = plan['slot_block'][c]
        own = nf_pad.reshape(-1, P, H)[blocks]          # [S, 128, 96]
        own_flat = own.reshape(S * P, H)
        nfbT = np.zeros((97, S * P), np.float32)
        nfbT[:H] = own_flat.T
        nfbT[96] = 1.0

        canon = plan['canon_edge'][c]
        real = canon >= 0
        ef_can = np.zeros((E_pad, H), np.float32)
        ef_can[real] = edge_feats[canon[real]]
        ef_pm = ef_can.reshape(TT, P, H).transpose(1, 0, 2).reshape(P, TT * H)
        nfb_pm = own.transpose(1, 0, 2).reshape(P, S * H)

        dstloc = np.full(E_pad, -1.0, np.float32)
        dstloc[real] = (dst[canon[real]] % P).astype(np.float32)
        dstloc = dstloc.reshape(TT, P).T.copy()         # [128, TT]

        gpos = np.zeros(E_pad, np.int64)
        gpos[real] = plan['src_pos'][c, canon[real]] % plan['WSZ']
        gidx = np.zeros((16, E_pad // 16), np.int16)
        idx_lin = np.arange(E_pad)
        gidx[idx_lin % 16, idx_lin // 16] = gpos.astype(np.int16)
        gidx = np.tile(gidx, (8, 1))                    # [128, E_pad/16]

        in_maps.append({
            'nftc': nftc.astype(BF16),
            'nfbT': nfbT.astype(BF16),
            'w1b': W1b.astype(BF16), 'w2b': W2b.astype(BF16),
            'weg': np.asarray(inputs['W_eg'], np.float32).astype(BF16),
            'efT': ef_can.T.astype(BF16).copy(),
            'ef_pm': ef_pm.astype(BF16),
            'dstloc': dstloc,
            'gidx': gidx,
            'iota': iota,
            'ident': ident.astype(BF16),
            'nfb': nfb_pm,
        })
    return in_maps


# ----------------------------------------------------------------------------
# device kernel
# ----------------------------------------------------------------------------

def build_kernel(plan):
    import concourse.bacc as bacc
    import concourse.bass as bass
    import concourse.mybir as mybir
    import concourse.tile as tile

    f32, bf16, i16 = mybir.dt.float32, mybir.dt.bfloat16, mybir.dt.int16
    AF = mybir.ActivationFunctionType
    ALU = mybir.AluOpType

    S, E_TBL, E_pad, TT = plan['S'], plan['E_TBL'], plan['E_pad'], plan['TT']
    n_win, WSZ = plan['n_win'], plan['WSZ']
    sched = plan['sched']
    NB = S * P

    nc = bacc.Bacc()
    dp = nc.declare_dram_parameter
    nftc = dp('nftc', [97, E_TBL], bf16, isOutput=False)
    nfbT = dp('nfbT', [97, NB], bf16, isOutput=False)
    w1b = dp('w1b', [97, 192], bf16, isOutput=False)
    w2b = dp('w2b', [97, 192], bf16, isOutput=False)
    weg = dp('weg', [H, H], bf16, isOutput=False)
    efT = dp('efT', [H, E_pad], bf16, isOutput=False)
    ef_pm = dp('ef_pm', [P, TT * H], bf16, isOutput=False)
    dstloc = dp('dstloc', [P, TT], f32, isOutput=False)
    gidx = dp('gidx', [P, E_pad // 16], i16, isOutput=False)
    iota = dp('iota', [P, P], f32, isOutput=False)
    ident = dp('ident', [P, P], bf16, isOutput=False)
    nfb = dp('nfb', [P, S * H], f32, isOutput=False)
    y_pm = dp('y_pm', [P, TT * H], bf16, isOutput=True)
    xout = dp('xout', [P, S * H], f32, isOutput=True)

    t1c = nc.dram_tensor('t1c', [E_TBL, 256], bf16)
    t2x = nc.dram_tensor('t2x', [P, S * 192], bf16)

    with tile.TileContext(nc) as tc:
        with (
            tc.tile_pool(name='const', bufs=1) as cpool,
            tc.tile_pool(name='io', bufs=4) as iop,
            tc.tile_pool(name='pa', bufs=2) as pa,
            tc.tile_pool(name='eft', bufs=2) as efp,
            tc.tile_pool(name='msb', bufs=10) as msp,
            tc.tile_pool(name='work', bufs=4) as wk,
            tc.tile_pool(name='grp', bufs=2) as grp,
            tc.tile_pool(name='yb', bufs=2) as ybp,
            tc.tile_pool(name='ps', bufs=3, space='PSUM') as pp,
            tc.tile_pool(name='ps_sum', bufs=2, space='PSUM') as pps,
        ):
            # ---- constants ----
            iota_sb = cpool.tile([P, P], f32, tag='iota')
            nc.sync.dma_start(out=iota_sb[:], in_=iota[:])
            id_bf = cpool.tile([P, P], bf16, tag='idb')
            nc.sync.dma_start(out=id_bf[:], in_=ident[:])
            w1_sb = cpool.tile([97, 192], bf16, tag='w1')
            nc.sync.dma_start(out=w1_sb[:], in_=w1b[:])
            w2_sb = cpool.tile([97, 192], bf16, tag='w2')
            nc.sync.dma_start(out=w2_sb[:], in_=w2b[:])
            weg_sb = cpool.tile([H, H], bf16, tag='weg')
            nc.sync.dma_start(out=weg_sb[:], in_=weg[:])
            idx_all = cpool.tile([P, E_pad // 16], i16, tag='gidx')
            nc.sync.dma_start(out=idx_all[:], in_=gidx[:])
            dl_all = cpool.tile([P, TT], f32, tag='dstloc')
            nc.sync.dma_start(out=dl_all[:], in_=dstloc[:])
            eps_col = cpool.tile([P, 1], f32, tag='eps')
            nc.vector.memset(eps_col[:], LN_EPS)
            xpre_all = cpool.tile([P, S * H], bf16, tag='xpre')

            # ---- phase A: node transform tables ----
            ACH = 16
            for (mode, srcT, wsb, n_tiles, dcols) in (
                ('t1', nftc, w1_sb, E_TBL // P, 256),
                ('t2', nfbT, w2_sb, S, 192),
            ):
                for j0 in range(0, n_tiles, ACH):
                    jn = min(ACH, n_tiles - j0)
                    nchunk = pa.tile([97, ACH * P], bf16, tag='nfa')
                    nc.sync.dma_start(
                        out=nchunk[:, :jn * P],
                        in_=srcT[:, j0 * P:(j0 + jn) * P])
                    tbuf = pa.tile([P, ACH * 256], bf16, tag='tbuf')
                    if mode == 't1':
                        nc.vector.memset(
                            tbuf[:].rearrange(
                                'p (j d) -> p j d', d=256)[:, :, 192:256], 0)
                    for k in range(jn):
                        mm = pp.tile([P, 192], f32, space='PSUM', tag='mm')
                        nc.tensor.matmul(
                            out=mm[:], lhsT=nchunk[:, k * P:(k + 1) * P],
                            rhs=wsb[:], start=True, stop=True)
                        nc.vector.tensor_copy(
                            out=tbuf[:, k * dcols:k * dcols + 192], in_=mm[:])
                    if mode == 't1':
                        nc.sync.dma_start(
                            out=t1c[j0 * P:(j0 + jn) * P, :].rearrange(
                                '(j p) d -> p j d', p=P),
                            in_=tbuf[:, :jn * 256].rearrange(
                                'p (j d) -> p j d', d=256))
                    else:
                        nc.sync.dma_start(
                            out=t2x[:, j0 * 192:(j0 + jn) * 192],
                            in_=tbuf[:, :jn * 192])

            # ---- phase B ----
            slot_first = {s: True for s in range(S)}
            slot_last = {}
            for (s, w, t, off) in sched:
                slot_last[s] = off + t - 1

            pending = []     # (off, t, msb, efg) per (s,w) group
            pend_n = [0]
            stats_buf = [None]

            def ln_coeffs(st, g):
                """Batched LN: stats [P, g, 6] -> (rstd, nmr) [P, g]."""
                stv = st[:].rearrange('p (g s) -> p g s', s=6)
                a1 = grp.tile([P, DG], f32, tag='a1')
                nc.vector.tensor_add(
                    out=a1[:, :g], in0=stv[:, :g, 2], in1=stv[:, :g, 5])
                a2 = grp.tile([P, DG], f32, tag='a2')
                nc.vector.tensor_sub(
                    out=a2[:, :g], in0=stv[:, :g, 1], in1=stv[:, :g, 4])
                a3 = grp.tile([P, DG], f32, tag='a3')
                nc.vector.tensor_mul(
                    out=a3[:, :g], in0=a2[:, :g], in1=a2[:, :g])
                var = grp.tile([P, DG], f32, tag='var')
                nc.vector.tensor_scalar(
                    out=var[:, :g], in0=a1[:, :g], scalar1=1.0 / 96.0,
                    scalar2=None, op0=ALU.mult)
                nc.vector.tensor_scalar(
                    out=a3[:, :g], in0=a3[:, :g], scalar1=0.25,
                    scalar2=None, op0=ALU.mult)
                nc.vector.tensor_add(
                    out=var[:, :g], in0=var[:, :g], in1=a3[:, :g])
                std = grp.tile([P, DG], f32, tag='std')
                nc.scalar.activation(
                    out=std[:, :g], in_=var[:, :g], func=AF.Sqrt,
                    bias=eps_col[:])
                rstd = grp.tile([P, DG], f32, tag='rstd')
                nc.vector.reciprocal(out=rstd[:, :g], in_=std[:, :g])
                msum = grp.tile([P, DG], f32, tag='msum')
                nc.vector.tensor_add(
                    out=msum[:, :g], in0=stv[:, :g, 1], in1=stv[:, :g, 4])
                nmr = grp.tile([P, DG], f32, tag='nmr')
                nc.vector.tensor_mul(
                    out=nmr[:, :g], in0=msum[:, :g], in1=rstd[:, :g])
                nc.vector.tensor_scalar(
                    out=nmr[:, :g], in0=nmr[:, :g], scalar1=-0.5,
                    scalar2=None, op0=ALU.mult)
                return rstd, nmr

            def flush():
                if not pending:
                    return
                g = pend_n[0]
                rstd, nmr = ln_coeffs(stats_buf[0], g)
                ybuf = ybp.tile([P, DG * H], bf16, tag='ybuf')
                j = 0
                off0 = pending[0][0]
                for (off_, t_, msb_, efg_) in pending:
                    for k in range(t_):
                        nc.scalar.activation(
                            out=ybuf[:, (j + k) * H:(j + k + 1) * H],
                            in_=msb_[:, k * H:(k + 1) * H],
                            func=AF.Silu,
                            bias=nmr[:, j + k:j + k + 1],
                            scale=rstd[:, j + k:j + k + 1])
                    nc.vector.tensor_add(
                        out=ybuf[:, j * H:(j + t_) * H],
                        in0=ybuf[:, j * H:(j + t_) * H],
                        in1=efg_)
                    j += t_
                nc.sync.dma_start(
                    out=y_pm[:, off0 * H:(off0 + g) * H],
                    in_=ybuf[:, :g * H])
                pending.clear()
                pend_n[0] = 0
                stats_buf[0] = None

            cur_slot = -1
            cur_span = None
            t2base = 0
            sums = None
            win_base = [None]
            win_len = [0]
            win_eftg = [None]
            win_efg = [None]
            for (s, w, t, off) in sched:
                if pend_n[0] + t > DG:
                    flush()
                if s != cur_slot:
                    if s % 8 == 0:
                        t2span = iop.tile([P, 8 * 192], bf16, tag='t2span')
                        sn = min(8, S - s)
                        nc.sync.dma_start(
                            out=t2span[:, :sn * 192],
                            in_=t2x[:, s * 192:(s + sn) * 192])
                        cur_span = t2span
                    cur_slot = s
                    t2base = (s % 8) * 192
                    sums = pps.tile([P, 192], f32, space='PSUM', tag='sums')
                wrows = min(WSZ, E_TBL - w * WSZ)
                gbuf = wk.tile([P, t * 256], bf16, tag='gbuf')
                nc.gpsimd.dma_gather(
                    out_ap=gbuf[:].rearrange('p (t d) -> p t d', t=t),
                    in_ap=t1c[w * WSZ:w * WSZ + wrows, :],
                    idxs_ap=idx_all[:, off * 8:(off + t) * 8],
                    num_idxs=t * P,
                    num_idxs_reg=t * P,
                    elem_size=256,
                    single_packet=(t * P <= 512),
                )
                if win_base[0] is None or off >= win_base[0] + win_len[0]:
                    wb = off
                    wl = 0
                    for (s2, w2, t2, off2) in sched:
                        if off2 < wb:
                            continue
                        if wl + t2 > DG:
                            break
                        wl += t2
                    win_base[0] = wb
                    win_len[0] = wl
                    eftg_w = efp.tile([H, DG * P], bf16, tag='eftgw')
                    nc.sync.dma_start(
                        out=eftg_w[:, :wl * P],
                        in_=efT[:, wb * P:(wb + wl) * P])
                    efg_w = efp.tile([P, DG * H], bf16, tag='efgw')
                    nc.sync.dma_start(
                        out=efg_w[:, :wl * H],
                        in_=ef_pm[:, wb * H:(wb + wl) * H])
                    win_eftg[0] = eftg_w
                    win_efg[0] = efg_w
                lo = off - win_base[0]

                # batched one-hot for the group
                onehot = wk.tile([P, t * P], bf16, tag='onehot')
                nc.vector.tensor_tensor(
                    out=onehot[:].rearrange('p (t q) -> p t q', q=P),
                    in0=dl_all[:, off:off + t, None].to_broadcast([P, t, P]),
                    in1=iota_sb[:, None, :].to_broadcast([P, t, P]),
                    op=ALU.is_equal)
                trps = pp.tile([P, t * P], bf16, space='PSUM', tag='tr')
                for k in range(t):
                    nc.tensor.transpose(
                        out=trps[:, k * P:(k + 1) * P],
                        in_=onehot[:, k * P:(k + 1) * P],
                        identity=id_bf[:])
                ohne = wk.tile([P, t * P], bf16, tag='ohne')
                nc.vector.tensor_copy(out=ohne[:], in_=trps[:])

                mp = pp.tile([P, t * H], f32, space='PSUM', tag='mm')
                for k in range(t):
                    nc.tensor.matmul(
                        out=mp[:, k * H:(k + 1) * H],
                        lhsT=win_eftg[0][:, (lo + k) * P:(lo + k + 1) * P],
                        rhs=weg_sb[:], start=True, stop=False)
                    nc.tensor.matmul(
                        out=mp[:, k * H:(k + 1) * H],
                        lhsT=ohne[:, k * P:(k + 1) * P],
                        rhs=cur_span[:, t2base:t2base + H],
                        start=False, stop=True)
                msb = msp.tile([P, t * H], f32, tag='msb')
                nc.vector.tensor_add(
                    out=msb[:].rearrange('p (t f) -> p t f', f=H),
                    in0=mp[:].rearrange('p (t f) -> p t f', f=H),
                    in1=gbuf[:].rearrange('p (t d) -> p t d', d=256)[:, :, 0:H])

                valcat = wk.tile([P, t * 192], bf16, tag='valcat')
                vv = valcat[:].rearrange('p (t d) -> p t d', d=192)
                nc.scalar.activation(
                    out=vv[:, :, 0:H],
                    in_=msb[:].rearrange('p (t f) -> p t f', f=H),
                    func=AF.Sigmoid)
                nc.vector.tensor_tensor(
                    out=vv[:, :, H:192],
                    in0=gbuf[:].rearrange('p (t d) -> p t d', d=256)[:, :, H:192],
                    in1=vv[:, :, 0:H], op=ALU.mult)

                for k in range(t):
                    tt = off + k
                    nc.tensor.matmul(
                        out=sums[:],
                        lhsT=onehot[:, k * P:(k + 1) * P],
                        rhs=valcat[:, k * 192:(k + 1) * 192],
                        start=slot_first[s],
                        stop=(tt == slot_last[s]))
                    slot_first[s] = False

                if stats_buf[0] is None:
                    st_new = grp.tile([P, DG * 6], f32, tag='stats')
                    stats_buf[0] = st_new
                j = pend_n[0]
                for k0 in range(t):
                    nc.vector.bn_stats(
                        out=stats_buf[0][:, (j + k0) * 6:(j + k0 + 1) * 6],
                        in_=msb[:, k0 * H:(k0 + 1) * H])
                pending.append((off, t, msb,
                                win_efg[0][:, lo * H:(lo + t) * H]))
                pend_n[0] = j + t

                if off + t - 1 == slot_last[s]:
                    # xpre for slot s (LN deferred to final phase)
                    ssd = wk.tile([P, H], f32, tag='ssd')
                    nc.vector.tensor_scalar_add(
                        out=ssd[:], in0=sums[:, 0:H], scalar1=1e-6)
                    rec = wk.tile([P, H], f32, tag='rec')
                    nc.vector.reciprocal(out=rec[:], in_=ssd[:])
                    h = wk.tile([P, H], f32, tag='h')
                    nc.vector.tensor_mul(
                        out=h[:], in0=sums[:, H:192], in1=rec[:])
                    nc.vector.tensor_add(
                        out=xpre_all[:, s * H:(s + 1) * H],
                        in0=h[:], in1=cur_span[:, t2base + H:t2base + 192])
            flush()

            # ---- final phase: node LN + silu + residual ----
            for s0 in range(0, S, DG):
                g = min(DG, S - s0)
                st = grp.tile([P, DG * 6], f32, tag='stats')
                for k0 in range(g):
                    nc.vector.bn_stats(
                        out=st[:, k0 * 6:(k0 + 1) * 6],
                        in_=xpre_all[:, (s0 + k0) * H:(s0 + k0 + 1) * H])
                rstd, nmr = ln_coeffs(st, g)
                nfblk = ybp.tile([P, DG * H], f32, tag='nfblk')
                nc.sync.dma_start(
                    out=nfblk[:, :g * H],
                    in_=nfb[:, s0 * H:(s0 + g) * H])
                xbuf = ybp.tile([P, DG * H], f32, tag='ybuf')
                for k in range(g):
                    s = s0 + k
                    nc.scalar.activation(
                        out=xbuf[:, k * H:(k + 1) * H],
                        in_=xpre_all[:, s * H:(s + 1) * H],
                        func=AF.Silu, bias=nmr[:, k:k + 1],
                        scale=rstd[:, k:k + 1])
                nc.vector.tensor_add(
                    out=xbuf[:, :g * H], in0=xbuf[:, :g * H],
                    in1=nfblk[:, :g * H])
                nc.sync.dma_start(
                    out=xout[:, s0 * H:(s0 + g) * H],
                    in_=xbuf[:, :g * H])

    nc.finalize()
    return nc


# ----------------------------------------------------------------------------
# top-level
# ----------------------------------------------------------------------------

_TRACE = [False]


def kernel(**inputs):
    from concourse.bass_utils import run_bass_kernel_spmd

    src = np.asarray(inputs['src'])
    dst = np.asarray(inputs['dst'])
    node_feats = np.asarray(inputs['node_feats'], np.float32)
    edge_feats = np.asarray(inputs['edge_feats'], np.float32)
    N, E = node_feats.shape[0], edge_feats.shape[0]

    plan = build_plan(src, dst, N)
    in_maps = build_inputs(plan, inputs)
    nc = build_kernel(plan)
    res = run_bass_kernel_spmd(
        nc, in_maps, core_ids=list(range(NCORES)), trace=_TRACE[0])
    kernel.last_result = res

    x = np.zeros((N, H), np.float32)
    y = np.zeros((E, H), np.float32)
    for c in range(NCORES):
        out = res.results[c]
        blocks = plan['slot_block'][c]
        xs = out['xout'].reshape(P, plan['S'], H).transpose(1, 0, 2)
        for s_i, b in enumerate(blocks):
            lo = b * P
            hi = min(lo + P, N)
            if lo < N:
                x[lo:hi] = xs[s_i, :hi - lo]
        canon = plan['canon_edge'][c]
        real = canon >= 0
        y_can = np.asarray(out['y_pm']).reshape(
            P, plan['TT'], H).transpose(1, 0, 2).reshape(plan['E_pad'], H)
        y[canon[real]] = y_can[real].astype(np.float32)
    return x, y


# revision 25
# speedup vs baseline: 4.0774x; 1.0133x over previous
"""ALIGNN edge-gated message passing on 8 Trainium2 NeuronCores.

Strategy: edges partitioned by dst-block across cores (no collectives).
Each core receives host-prepared, per-core data:
  - a compacted node table (only nodes referenced as src by its edges)
  - its own 98 node-blocks (128 nodes each) in load-balanced "slot" order
  - its edge slice in canonical (slot, window, tile) order
Device work per core:
  phase A : PE matmuls node_feats -> T1=[e_src+bias | Bh] (f32, compact rows)
            and T2x=[e_dst | x_lin] (bf16, own blocks)
  phase B : per (slot,window) group of 128-edge tiles: dma_gather T1 rows by
            src (int16 windows), one-hot(dst_local) via batched is_equal,
            m = ef@W_eg + onehot.T@T2blk (+ gathered), sigma = sigmoid(m),
            segment-sum via one-hot matmul accumulation in PSUM;
            LN sqrt batched over deferred groups of 32 tiles, then
            y = ef + silu(LN(m)) written edge-major
  phase D : per slot: xpre = x_lin + ssh/(ss+1e-6); final phase applies
            LN + silu + residual for all slots with batched stats
"""
import sys

if '/opt/trn_rl_repo' not in sys.path:
    sys.path.insert(0, '/opt/trn_rl_repo')

import numpy as np
import ml_dtypes

BF16 = ml_dtypes.bfloat16
H = 96
LN_EPS = 1e-5
NCORES = 8
P = 128
WMAX = 32640          # gather window rows (<= int16 max, mult of 128)
DG = 32               # deferred-LN group size (tiles)


# ----------------------------------------------------------------------------
# host-side plan
# ----------------------------------------------------------------------------

def build_plan(src, dst, N):
    E = src.shape[0]
    n_blocks_real = (N + P - 1) // P
    n_blocks = ((n_blocks_real + NCORES - 1) // NCORES) * NCORES
    S = n_blocks // NCORES              # slots per core
    N_pad = n_blocks * P

    blk_of_edge = dst // P
    blk_counts = np.bincount(blk_of_edge, minlength=n_blocks)

    # balanced assignment: sort blocks by count desc, greedily fill cores
    order = np.argsort(-blk_counts, kind='stable')
    core_load = np.zeros(NCORES, dtype=np.int64)
    core_nblk = np.zeros(NCORES, dtype=np.int64)
    blk_core = np.zeros(n_blocks, dtype=np.int64)
    for b in order:
        cands = np.where(core_nblk < S)[0]
        c = cands[np.argmin(core_load[cands])]
        blk_core[b] = c
        core_load[c] += blk_counts[b]
        core_nblk[c] += 1

    # per-core slot order: blocks sorted by count desc
    slot_block = np.zeros((NCORES, S), dtype=np.int64)   # slot -> block id
    for c in range(NCORES):
        blks = np.where(blk_core == c)[0]
        blks = blks[np.argsort(-blk_counts[blks], kind='stable')]
        slot_block[c] = blks

    edge_core = blk_core[blk_of_edge]
    slot_of_block = np.zeros(n_blocks, dtype=np.int64)
    for c in range(NCORES):
        slot_of_block[slot_block[c]] = np.arange(S)
    edge_slot = slot_of_block[blk_of_edge]

    # compacted src table per core
    srclist = []
    for c in range(NCORES):
        u = np.unique(src[edge_core == c])
        srclist.append(u)
    E_TBL = ((max(len(u) for u in srclist) + P - 1) // P) * P
    n_win = max(1, (E_TBL + WMAX - 1) // WMAX)
    WSZ = ((E_TBL // n_win + P - 1) // P) * P
    assert WSZ <= 32767

    src_pos = np.zeros((NCORES, E), dtype=np.int64)
    for c in range(NCORES):
        m = edge_core == c
        src_pos[c, m] = np.searchsorted(srclist[c], src[m])

    cnt = np.zeros((NCORES, S, n_win), dtype=np.int64)
    for c in range(NCORES):
        m = edge_core == c
        w = src_pos[c, m] // WSZ
        np.add.at(cnt[c], (edge_slot[m], w), 1)
    tiles_sw = np.maximum(np.ceil(cnt / P).astype(np.int64).max(axis=0), 0)
    tiles_sw[:, 0] = np.maximum(tiles_sw[:, 0], 1)   # every slot >=1 tile
    TT = int(tiles_sw.sum())
    E_pad = TT * P

    sched = []
    off = 0
    TMAXG = 5
    for s in range(S):
        for w in range(n_win):
            t = int(tiles_sw[s, w])
            base = 0
            while t > 0:
                tc_ = min(t, TMAXG)
                # real edges of this call across cores
                mx = 0
                for c in range(NCORES):
                    rc = min(max(cnt[c, s, w] - base * P, 0), tc_ * P)
                    mx = max(mx, int(rc))
                ni = min(((mx + 15) // 16) * 16, tc_ * P)
                ni = max(ni, (tc_ - 1) * P + 16)
                sched.append((s, w, tc_, off, ni))
                off += tc_
                t -= tc_
                base += tc_

    canon_edge = np.full((NCORES, E_pad), -1, dtype=np.int64)
    for c in range(NCORES):
        m = np.where(edge_core == c)[0]
        w = src_pos[c, m] // WSZ
        key = edge_slot[m] * n_win + w
        ordr = np.argsort(key, kind='stable')
        me, ke = m[ordr], key[ordr]
        group_off = {}
        for (s_, w_, t_, o_, ni_) in sched:
            if (s_, w_) not in group_off:
                group_off[(s_, w_)] = o_ * P
        pos = np.zeros(len(me), dtype=np.int64)
        start = 0
        for k in np.unique(ke):
            cnt_k = int((ke == k).sum())
            s_, w_ = divmod(int(k), n_win)
            base = group_off[(s_, w_)]
            pos[start:start + cnt_k] = base + np.arange(cnt_k)
            start += cnt_k
        canon_edge[c, pos] = me
    return dict(
        N_pad=N_pad, n_blocks=n_blocks, S=S, E_TBL=E_TBL, n_win=n_win,
        WSZ=WSZ, TT=TT, E_pad=E_pad, sched=sched, slot_block=slot_block,
        srclist=srclist, src_pos=src_pos, canon_edge=canon_edge,
    )


def build_inputs(plan, inputs):
    node_feats = np.asarray(inputs['node_feats'], np.float32)
    edge_feats = np.asarray(inputs['edge_feats'], np.float32)
    src = np.asarray(inputs['src'])
    dst = np.asarray(inputs['dst'])
    N = node_feats.shape[0]

    tp = (np.asarray(inputs['time_feats'], np.float32) @
          np.asarray(inputs['W_tp'], np.float32) +
          np.asarray(inputs['b_tp'], np.float32))[0]
    bias_src = np.asarray(inputs['b_sg'], np.float32) + tp + \
        np.asarray(inputs['b_eg'], np.float32)

    W1b = np.concatenate([
        np.concatenate([inputs['W_sg'], inputs['W_du']], axis=1),
        np.concatenate([bias_src, inputs['b_du']])[None, :],
    ], axis=0).astype(np.float32)                      # [97, 192]
    W2b = np.concatenate([
        np.concatenate([inputs['W_dg'], inputs['W_su']], axis=1),
        np.concatenate([inputs['b_dg'], inputs['b_su']])[None, :],
    ], axis=0).astype(np.float32)                      # [97, 192]

    S, E_TBL, E_pad, TT = plan['S'], plan['E_TBL'], plan['E_pad'], plan['TT']
    nf_pad = np.zeros((plan['N_pad'], H), np.float32)
    nf_pad[:N] = node_feats

    iota = np.tile(np.arange(P, dtype=np.float32), (P, 1))
    ident = np.eye(P, dtype=np.float32)

    in_maps = []
    for c in range(NCORES):
        u = plan['srclist'][c]
        nftc = np.zeros((97, E_TBL), np.float32)
        nftc[:H, :len(u)] = node_feats[u].T
        nftc[96, :] = 1.0

        blocks = plan['slot_block'][c]
        own = nf_pad.reshape(-1, P, H)[blocks]          # [S, 128, 96]
        own_flat = own.reshape(S * P, H)
        nfbT = np.zeros((97, S * P), np.float32)
        nfbT[:H] = own_flat.T
        nfbT[96] = 1.0

        canon = plan['canon_edge'][c]
        real = canon >= 0
        ef_can = np.zeros((E_pad, H), np.float32)
        ef_can[real] = edge_feats[canon[real]]
        ef_pm = ef_can.reshape(TT, P, H).transpose(1, 0, 2).reshape(P, TT * H)
        nfb_pm = own.transpose(1, 0, 2).reshape(P, S * H)

        dstloc = np.full(E_pad, -1.0, np.float32)
        dstloc[real] = (dst[canon[real]] % P).astype(np.float32)
        dstloc = dstloc.reshape(TT, P).T.copy()         # [128, TT]

        gpos = np.zeros(E_pad, np.int64)
        gpos[real] = plan['src_pos'][c, canon[real]] % plan['WSZ']
        gidx = np.zeros((16, E_pad // 16), np.int16)
        idx_lin = np.arange(E_pad)
        gidx[idx_lin % 16, idx_lin // 16] = gpos.astype(np.int16)
        gidx = np.tile(gidx, (8, 1))                    # [128, E_pad/16]

        in_maps.append({
            'nftc': nftc.astype(BF16),
            'nfbT': nfbT.astype(BF16),
            'w1b': W1b.astype(BF16), 'w2b': W2b.astype(BF16),
            'weg': np.asarray(inputs['W_eg'], np.float32).astype(BF16),
            'efT': ef_can.T.astype(BF16).copy(),
            'ef_pm': ef_pm.astype(BF16),
            'dstloc': dstloc,
            'gidx': gidx,
            'iota': iota,
            'ident': ident.astype(BF16),
            'nfb': nfb_pm,
        })
    return in_maps


# ----------------------------------------------------------------------------
# device kernel
# ----------------------------------------------------------------------------

def build_kernel(plan):
    import concourse.bacc as bacc
    import concourse.bass as bass
    import concourse.mybir as mybir
    import concourse.tile as tile

    f32, bf16, i16 = mybir.dt.float32, mybir.dt.bfloat16, mybir.dt.int16
    AF = mybir.ActivationFunctionType
    ALU = mybir.AluOpType

    S, E_TBL, E_pad, TT = plan['S'], plan['E_TBL'], plan['E_pad'], plan['TT']
    n_win, WSZ = plan['n_win'], plan['WSZ']
    sched = plan['sched']
    NB = S * P

    nc = bacc.Bacc()
    dp = nc.declare_dram_parameter
    nftc = dp('nftc', [97, E_TBL], bf16, isOutput=False)
    nfbT = dp('nfbT', [97, NB], bf16, isOutput=False)
    w1b = dp('w1b', [97, 192], bf16, isOutput=False)
    w2b = dp('w2b', [97, 192], bf16, isOutput=False)
    weg = dp('weg', [H, H], bf16, isOutput=False)
    efT = dp('efT', [H, E_pad], bf16, isOutput=False)
    ef_pm = dp('ef_pm', [P, TT * H], bf16, isOutput=False)
    dstloc = dp('dstloc', [P, TT], f32, isOutput=False)
    gidx = dp('gidx', [P, E_pad // 16], i16, isOutput=False)
    iota = dp('iota', [P, P], f32, isOutput=False)
    ident = dp('ident', [P, P], bf16, isOutput=False)
    nfb = dp('nfb', [P, S * H], f32, isOutput=False)
    y_pm = dp('y_pm', [P, TT * H], bf16, isOutput=True)
    xout = dp('xout', [P, S * H], f32, isOutput=True)

    t1cw = []
    for w in range(n_win):
        wr = min(WSZ, E_TBL - w * WSZ)
        t1cw.append(nc.dram_tensor(f't1c{w}', [wr, 256], bf16))
    t2x = nc.dram_tensor('t2x', [P, S * 192], bf16)

    with tile.TileContext(nc) as tc:
        with (
            tc.tile_pool(name='const', bufs=1) as cpool,
            tc.tile_pool(name='io', bufs=4) as iop,
            tc.tile_pool(name='pa', bufs=2) as pa,
            tc.tile_pool(name='eft', bufs=2) as efp,
            tc.tile_pool(name='msb', bufs=10) as msp,
            tc.tile_pool(name='work', bufs=4) as wk,
            tc.tile_pool(name='grp', bufs=2) as grp,
            tc.tile_pool(name='yb', bufs=2) as ybp,
            tc.tile_pool(name='ps', bufs=3, space='PSUM') as pp,
            tc.tile_pool(name='ps_sum', bufs=2, space='PSUM') as pps,
        ):
            # ---- constants ----
            iota_sb = cpool.tile([P, P], f32, tag='iota')
            nc.sync.dma_start(out=iota_sb[:], in_=iota[:])
            id_bf = cpool.tile([P, P], bf16, tag='idb')
            nc.sync.dma_start(out=id_bf[:], in_=ident[:])
            w1_sb = cpool.tile([97, 192], bf16, tag='w1')
            nc.sync.dma_start(out=w1_sb[:], in_=w1b[:])
            w2_sb = cpool.tile([97, 192], bf16, tag='w2')
            nc.sync.dma_start(out=w2_sb[:], in_=w2b[:])
            weg_sb = cpool.tile([H, H], bf16, tag='weg')
            nc.sync.dma_start(out=weg_sb[:], in_=weg[:])
            idx_all = cpool.tile([P, E_pad // 16], i16, tag='gidx')
            nc.sync.dma_start(out=idx_all[:], in_=gidx[:])
            dl_all = cpool.tile([P, TT], f32, tag='dstloc')
            nc.sync.dma_start(out=dl_all[:], in_=dstloc[:])
            eps_col = cpool.tile([P, 1], f32, tag='eps')
            nc.vector.memset(eps_col[:], LN_EPS)
            xpre_all = cpool.tile([P, S * H], bf16, tag='xpre')

            # ---- phase A: node transform tables ----
            ACH = 16
            phase_a = [('t2', nfbT, w2_sb, 0, S, 192, None)]
            for w in range(n_win - 1, -1, -1):
                wr = min(WSZ, E_TBL - w * WSZ)
                phase_a.append(
                    ('t1', nftc, w1_sb, w * WSZ // P, wr // P, 256, w))
            # emit window 0 last?? no: emit t2 first, then w1.. wait
            phase_a = [phase_a[0]] + phase_a[:0:-1]
            for (mode, srcT, wsb, tile0, n_tiles, dcols, wid) in phase_a:
                for j0 in range(0, n_tiles, ACH):
                    jn = min(ACH, n_tiles - j0)
                    nchunk = pa.tile([97, ACH * P], bf16, tag='nfa')
                    nc.scalar.dma_start(
                        out=nchunk[:, :jn * P],
                        in_=srcT[:, (tile0 + j0) * P:(tile0 + j0 + jn) * P])
                    tbuf = pa.tile([P, ACH * 256], bf16, tag='tbuf')
                    if mode == 't1':
                        nc.vector.memset(
                            tbuf[:].rearrange(
                                'p (j d) -> p j d', d=256)[:, :, 192:256], 0)
                    for k in range(0, jn, 2):
                        kn = min(2, jn - k)
                        mm = pp.tile([P, 2 * 192], f32, space='PSUM', tag='mm')
                        for q in range(kn):
                            nc.tensor.matmul(
                                out=mm[:, q * 192:(q + 1) * 192],
                                lhsT=nchunk[:, (k + q) * P:(k + q + 1) * P],
                                rhs=wsb[:], start=True, stop=True)
                        nc.vector.tensor_copy(
                            out=tbuf[:, k * dcols:k * dcols + kn * dcols]
                            .rearrange('p (j d) -> p j d', d=dcols)[:, :, 0:192]
                            if dcols == 256 else
                            tbuf[:, k * dcols:(k + kn) * dcols],
                            in_=mm[:, :kn * 192].rearrange(
                                'p (j d) -> p j d', d=192)
                            if dcols == 256 else mm[:, :kn * 192])
                    if mode == 't1':
                        nc.sync.dma_start(
                            out=t1cw[wid][j0 * P:(j0 + jn) * P, :].rearrange(
                                '(j p) d -> p j d', p=P),
                            in_=tbuf[:, :jn * 256].rearrange(
                                'p (j d) -> p j d', d=256))
                    else:
                        nc.sync.dma_start(
                            out=t2x[:, j0 * 192:(j0 + jn) * 192],
                            in_=tbuf[:, :jn * 192])

            # ---- phase B ----
            slot_first = {s: True for s in range(S)}
            slot_last = {}
            for (s, w, t, off, ni) in sched:
                slot_last[s] = off + t - 1
            # DIY gather ring (stale-safe: memset once)
            TMAXG = 5
            gring = []
            for r in range(4):
                gt = cpool.tile([P, TMAXG * 256], bf16, tag=f'gring{r}')
                nc.vector.memset(gt[:], 0)
                gring.append(gt)
            gring_i = [0]

            pending = []     # (off, t, msb, efg) per (s,w) group
            pend_n = [0]
            stats_buf = [None]

            def ln_coeffs(st, g):
                """Batched LN: stats [P, g, 6] -> (rstd, nmr) [P, g]."""
                stv = st[:].rearrange('p (g s) -> p g s', s=6)
                a1 = grp.tile([P, DG], f32, tag='a1')
                nc.vector.tensor_add(
                    out=a1[:, :g], in0=stv[:, :g, 2], in1=stv[:, :g, 5])
                a2 = grp.tile([P, DG], f32, tag='a2')
                nc.vector.tensor_sub(
                    out=a2[:, :g], in0=stv[:, :g, 1], in1=stv[:, :g, 4])
                a3 = grp.tile([P, DG], f32, tag='a3')
                nc.vector.tensor_mul(
                    out=a3[:, :g], in0=a2[:, :g], in1=a2[:, :g])
                var = grp.tile([P, DG], f32, tag='var')
                nc.vector.tensor_scalar(
                    out=var[:, :g], in0=a1[:, :g], scalar1=1.0 / 96.0,
                    scalar2=None, op0=ALU.mult)
                nc.vector.tensor_scalar(
                    out=a3[:, :g], in0=a3[:, :g], scalar1=0.25,
                    scalar2=None, op0=ALU.mult)
                nc.vector.tensor_add(
                    out=var[:, :g], in0=var[:, :g], in1=a3[:, :g])
                std = grp.tile([P, DG], f32, tag='std')
                nc.scalar.activation(
                    out=std[:, :g], in_=var[:, :g], func=AF.Sqrt,
                    bias=eps_col[:])
                rstd = grp.tile([P, DG], f32, tag='rstd')
                nc.vector.reciprocal(out=rstd[:, :g], in_=std[:, :g])
                msum = grp.tile([P, DG], f32, tag='msum')
                nc.vector.tensor_add(
                    out=msum[:, :g], in0=stv[:, :g, 1], in1=stv[:, :g, 4])
                nmr = grp.tile([P, DG], f32, tag='nmr')
                nc.vector.tensor_mul(
                    out=nmr[:, :g], in0=msum[:, :g], in1=rstd[:, :g])
                nc.vector.tensor_scalar(
                    out=nmr[:, :g], in0=nmr[:, :g], scalar1=-0.5,
                    scalar2=None, op0=ALU.mult)
                return rstd, nmr

            def flush():
                if not pending:
                    return
                g = pend_n[0]
                rstd, nmr = ln_coeffs(stats_buf[0], g)
                ybuf = ybp.tile([P, DG * H], bf16, tag='ybuf')
                j = 0
                off0 = pending[0][0]
                for (off_, t_, msb_, efg_) in pending:
                    for k in range(t_):
                        nc.scalar.activation(
                            out=ybuf[:, (j + k) * H:(j + k + 1) * H],
                            in_=msb_[:, k * H:(k + 1) * H],
                            func=AF.Silu,
                            bias=nmr[:, j + k:j + k + 1],
                            scale=rstd[:, j + k:j + k + 1])
                    nc.vector.tensor_add(
                        out=ybuf[:, j * H:(j + t_) * H],
                        in0=ybuf[:, j * H:(j + t_) * H],
                        in1=efg_)
                    j += t_
                nc.sync.dma_start(
                    out=y_pm[:, off0 * H:(off0 + g) * H],
                    in_=ybuf[:, :g * H])
                pending.clear()
                pend_n[0] = 0
                stats_buf[0] = None

            cur_slot = -1
            cur_span = None
            t2base = 0
            sums = None
            win_base = [None]
            win_len = [0]
            win_eftg = [None]
            win_efg = [None]
            for (s, w, t, off, ni) in sched:
                if pend_n[0] + t > DG:
                    flush()
                if s != cur_slot:
                    if s % 8 == 0:
                        t2span = iop.tile([P, 8 * 192], bf16, tag='t2span')
                        sn = min(8, S - s)
                        nc.sync.dma_start(
                            out=t2span[:, :sn * 192],
                            in_=t2x[:, s * 192:(s + sn) * 192])
                        cur_span = t2span
                    cur_slot = s
                    t2base = (s % 8) * 192
                    sums = pps.tile([P, 192], f32, space='PSUM', tag='sums')
                gbuf = gring[gring_i[0] % 4]
                gring_i[0] += 1
                nc.gpsimd.dma_gather(
                    out_ap=gbuf[:, :t * 256].rearrange(
                        'p (t d) -> p t d', t=t),
                    in_ap=t1cw[w][:],
                    idxs_ap=idx_all[:, off * 8:off * 8 + (ni + 15) // 16],
                    num_idxs=ni,
                    num_idxs_reg=ni,
                    elem_size=256,
                    single_packet=(ni <= 512),
                )
                if win_base[0] is None or off >= win_base[0] + win_len[0]:
                    wb = off
                    wl = 0
                    for (s2, w2, t2, off2, ni2) in sched:
                        if off2 < wb:
                            continue
                        if wl + t2 > DG:
                            break
                        wl += t2
                    win_base[0] = wb
                    win_len[0] = wl
                    eftg_w = efp.tile([H, DG * P], bf16, tag='eftgw')
                    nc.sync.dma_start(
                        out=eftg_w[:, :wl * P],
                        in_=efT[:, wb * P:(wb + wl) * P])
                    efg_w = efp.tile([P, DG * H], bf16, tag='efgw')
                    nc.sync.dma_start(
                        out=efg_w[:, :wl * H],
                        in_=ef_pm[:, wb * H:(wb + wl) * H])
                    win_eftg[0] = eftg_w
                    win_efg[0] = efg_w
                lo = off - win_base[0]

                # batched one-hot for the group
                onehot = wk.tile([P, t * P], bf16, tag='onehot')
                nc.vector.tensor_tensor(
                    out=onehot[:].rearrange('p (t q) -> p t q', q=P),
                    in0=dl_all[:, off:off + t, None].to_broadcast([P, t, P]),
                    in1=iota_sb[:, None, :].to_broadcast([P, t, P]),
                    op=ALU.is_equal)
                trps = pp.tile([P, t * P], bf16, space='PSUM', tag='tr')
                for k in range(t):
                    nc.tensor.transpose(
                        out=trps[:, k * P:(k + 1) * P],
                        in_=onehot[:, k * P:(k + 1) * P],
                        identity=id_bf[:])
                ohne = wk.tile([P, t * P], bf16, tag='ohne')
                nc.vector.tensor_copy(out=ohne[:], in_=trps[:])

                mp = pp.tile([P, t * H], f32, space='PSUM', tag='mm')
                for k in range(t):
                    nc.tensor.matmul(
                        out=mp[:, k * H:(k + 1) * H],
                        lhsT=win_eftg[0][:, (lo + k) * P:(lo + k + 1) * P],
                        rhs=weg_sb[:], start=True, stop=False)
                    nc.tensor.matmul(
                        out=mp[:, k * H:(k + 1) * H],
                        lhsT=ohne[:, k * P:(k + 1) * P],
                        rhs=cur_span[:, t2base:t2base + H],
                        start=False, stop=True)
                msb = msp.tile([P, t * H], f32, tag='msb')
                nc.vector.tensor_add(
                    out=msb[:].rearrange('p (t f) -> p t f', f=H),
                    in0=mp[:].rearrange('p (t f) -> p t f', f=H),
                    in1=gbuf[:, :t * 256].rearrange('p (t d) -> p t d', d=256)[:, :, 0:H])

                valcat = wk.tile([P, t * 192], bf16, tag='valcat')
                vv = valcat[:].rearrange('p (t d) -> p t d', d=192)
                nc.scalar.activation(
                    out=vv[:, :, 0:H],
                    in_=msb[:].rearrange('p (t f) -> p t f', f=H),
                    func=AF.Sigmoid)
                nc.vector.tensor_tensor(
                    out=vv[:, :, H:192],
                    in0=gbuf[:, :t * 256].rearrange('p (t d) -> p t d', d=256)[:, :, H:192],
                    in1=vv[:, :, 0:H], op=ALU.mult)

                for k in range(t):
                    tt = off + k
                    nc.tensor.matmul(
                        out=sums[:],
                        lhsT=onehot[:, k * P:(k + 1) * P],
                        rhs=valcat[:, k * 192:(k + 1) * 192],
                        start=slot_first[s],
                        stop=(tt == slot_last[s]))
                    slot_first[s] = False

                if stats_buf[0] is None:
                    st_new = grp.tile([P, DG * 6], f32, tag='stats')
                    stats_buf[0] = st_new
                j = pend_n[0]
                for k0 in range(t):
                    nc.vector.bn_stats(
                        out=stats_buf[0][:, (j + k0) * 6:(j + k0 + 1) * 6],
                        in_=msb[:, k0 * H:(k0 + 1) * H])
                pending.append((off, t, msb,
                                win_efg[0][:, lo * H:(lo + t) * H]))
                pend_n[0] = j + t

                if off + t - 1 == slot_last[s]:
                    # xpre for slot s (LN deferred to final phase)
                    ssd = wk.tile([P, H], f32, tag='ssd')
                    nc.vector.tensor_scalar_add(
                        out=ssd[:], in0=sums[:, 0:H], scalar1=1e-6)
                    rec = wk.tile([P, H], f32, tag='rec')
                    nc.vector.reciprocal(out=rec[:], in_=ssd[:])
                    h = wk.tile([P, H], f32, tag='h')
                    nc.vector.tensor_mul(
                        out=h[:], in0=sums[:, H:192], in1=rec[:])
                    nc.vector.tensor_add(
                        out=xpre_all[:, s * H:(s + 1) * H],
                        in0=h[:], in1=cur_span[:, t2base + H:t2base + 192])
            flush()

            # ---- final phase: node LN + silu + residual ----
            for s0 in range(0, S, DG):
                g = min(DG, S - s0)
                st = grp.tile([P, DG * 6], f32, tag='stats')
                for k0 in range(g):
                    nc.vector.bn_stats(
                        out=st[:, k0 * 6:(k0 + 1) * 6],
                        in_=xpre_all[:, (s0 + k0) * H:(s0 + k0 + 1) * H])
                rstd, nmr = ln_coeffs(st, g)
                nfblk = ybp.tile([P, DG * H], f32, tag='nfblk')
                nc.sync.dma_start(
                    out=nfblk[:, :g * H],
                    in_=nfb[:, s0 * H:(s0 + g) * H])
                xbuf = ybp.tile([P, DG * H], f32, tag='ybuf')
                for k in range(g):
                    s = s0 + k
                    nc.scalar.activation(
                        out=xbuf[:, k * H:(k + 1) * H],
                        in_=xpre_all[:, s * H:(s + 1) * H],
                        func=AF.Silu, bias=nmr[:, k:k + 1],
                        scale=rstd[:, k:k + 1])
                nc.vector.tensor_add(
                    out=xbuf[:, :g * H], in0=xbuf[:, :g * H],
                    in1=nfblk[:, :g * H])
                nc.sync.dma_start(
                    out=xout[:, s0 * H:(s0 + g) * H],
                    in_=xbuf[:, :g * H])

    nc.finalize()
    return nc


# ----------------------------------------------------------------------------
# top-level
# ----------------------------------------------------------------------------

_TRACE = [False]


def kernel(**inputs):
    from concourse.bass_utils import run_bass_kernel_spmd

    src = np.asarray(inputs['src'])
    dst = np.asarray(inputs['dst'])
    node_feats = np.asarray(inputs['node_feats'], np.float32)
    edge_feats = np.asarray(inputs['edge_feats'], np.float32)
    N, E = node_feats.shape[0], edge_feats.shape[0]

    plan = build_plan(src, dst, N)
    in_maps = build_inputs(plan, inputs)
    nc = build_kernel(plan)
    res = run_bass_kernel_spmd(
        nc, in_maps, core_ids=list(range(NCORES)), trace=_TRACE[0])
    kernel.last_result = res

    x = np.zeros((N, H), np.float32)
    y = np.zeros((E, H), np.float32)
    for c in range(NCORES):
        out = res.results[c]
        blocks = plan['slot_block'][c]
        xs = out['xout'].reshape(P, plan['S'], H).transpose(1, 0, 2)
        for s_i, b in enumerate(blocks):
            lo = b * P
            hi = min(lo + P, N)
            if lo < N:
                x[lo:hi] = xs[s_i, :hi - lo]
        canon = plan['canon_edge'][c]
        real = canon >= 0
        y_can = np.asarray(out['y_pm']).reshape(
            P, plan['TT'], H).transpose(1, 0, 2).reshape(plan['E_pad'], H)
        y[canon[real]] = y_can[real].astype(np.float32)
    return x, y
